# revision 5
# baseline (speedup 1.0000x reference)
"""Trainium2 Bass kernel for the ChebConv GNN problem
(nn_ChebConvConvolutional): 2x GCNConv + 1x ChebConv(K=3), N=10000 nodes,
E=160000 edges, F=512, celu activations.

Strategy (8 NeuronCores, SPMD):
  * Nodes are sharded 1250/core (padded to 1280). Edges are sharded by
    destination core and grouped into 128-dest tiles; per dest-tile the
    source nodes are deduplicated and the edge weights are baked into dense
    [128 src x 128 dst] one-hot "S" matrices (GCN self-loops folded in as
    edges with value dinv^2, Cheb normalization negated so the scatter
    directly produces lhat).
  * Pipelined AllGather: every aggregation's sources are split into two
    halves (local rows 0-639 / 640-1279 of each rank). The producing layer
    issues the AG of chunk 0 after its 5th dest tile, chunk 1 at the end;
    the consuming layer first processes all half-A source blocks (needs
    only chunk 0), then half-B, so collectives overlap compute instead of
    stalling the PE (which also avoids HAM cold-clock restarts).
  * ChebConv is computed transform-first:
        y1 = h2@W1, y2 = h2@W2, acc0 = h2@(W0-W2) + bc
        out = celu(acc0 + lhat(y1) + 2*lhat(lhat(y2)))
    so the second-hop aggregate and its AllGather are only 256 wide, and
    no node-major T_k tensors are materialized or transposed.
"""
import numpy as np
import ml_dtypes

import concourse.bacc as bacc
import concourse.mybir as mybir
import concourse.tile as tile
from concourse import library_config
from concourse.bass_utils import run_bass_kernel_spmd
from concourse.tile import add_dep_helper

BF16 = ml_dtypes.bfloat16
FP32 = mybir.dt.float32
BF16D = mybir.dt.bfloat16
I16 = mybir.dt.int16

P = 8            # cores
N = 10000        # nodes
NPC = N // P     # nodes per core
NPAD = 1280      # padded nodes per core
NTOT = NPAD * P
F = 512          # feature width of x / h1 / h2 / [y1|y2]
DOUT = 256
DT = 128         # dests per dest tile
NDT = NPAD // DT # dest tiles per core
KC = F // 128    # contraction chunks (4)
NCH = 2          # AllGather chunks per layer (source-split pipelining)
CH = NPAD // NCH # local rows per AG chunk (640)
HALF = P * CH    # global padded rows per source half (5120)


# ----------------------------------------------------------------- host prep

def _to_padded_id(n):
    """Global node id -> row in the chunked-AllGather global layout:
    [NCH chunks][P ranks][CH rows]."""
    r = n // NPC
    l = n % NPC
    j = l // CH
    return j * (P * CH) + r * CH + (l % CH)


def _build_edge_tiles_split(src, dst, val):
    """Shard by dest core, tile by 128 dests, dedup sources per tile, then
    split each tile's sources by AG half (padded id </>= HALF).
    Returns, for each half X in (A, B):
      ETX [NDT], idxX [P, TX, 128] int32 (ids relative to the half),
      SX [P, TX, 128, DT]."""
    order = np.argsort(dst, kind="stable")
    src, dst, val = src[order], dst[order], val[order]
    core_of = dst // NPC
    core_starts = np.searchsorted(core_of, np.arange(P + 1))
    per_core = []
    for c in range(P):
        lo, hi = core_starts[c], core_starts[c + 1]
        s, d, v = src[lo:hi], dst[lo:hi] - c * NPC, val[lo:hi]
        tile_of = d // DT
        tile_starts = np.searchsorted(tile_of, np.arange(NDT + 1))
        groups = []
        for t in range(NDT):
            a, b = tile_starts[t], tile_starts[t + 1]
            st, dl, vt = s[a:b], d[a:b] - t * DT, v[a:b]
            uniq, inv = np.unique(st, return_inverse=True)
            halves = []
            if len(uniq) == 0:
                halves = [(np.zeros(1, np.int64), np.zeros((1, DT), np.float32))] * 2
            else:
                S = np.zeros((len(uniq), DT), np.float32)
                np.add.at(S, (inv, dl), vt)
                pid = _to_padded_id(uniq)
                for h in range(2):
                    m = (pid < HALF) if h == 0 else (pid >= HALF)
                    if not np.any(m):
                        halves.append((np.zeros(1, np.int64),
                                       np.zeros((1, DT), np.float32)))
                    else:
                        halves.append((pid[m] - h * HALF, S[m]))
            groups.append(halves)
        per_core.append(groups)

    out = []
    for h in range(2):
        ET = [max(max((len(per_core[c][t][h][0]) + 127) // 128, 1)
                  for c in range(P)) for t in range(NDT)]
        T = sum(ET)
        off = np.cumsum([0] + ET[:-1])
        idx = np.zeros((P, T, 128), np.int32)
        S_all = np.zeros((P, T, 128, DT), np.float32)
        for c in range(P):
            for t in range(NDT):
                ids, S = per_core[c][t][h]
                n = len(ids)
                o = off[t]
                idx[c, o:o + (n + 127) // 128].reshape(-1)[:n] = ids
                S_all[c, o:o + (n + 127) // 128].reshape(-1, DT)[:n] = S
        out.append((tuple(ET), idx, S_all))
    return out


def _idx_dev(idx_core):
    """[T, 128] int32 -> [128, T*8] int16 (wrap 16 partitions, replicate x8)."""
    flat = idx_core.reshape(-1)
    n = len(flat)
    a = np.zeros((16, n // 16), np.int16)
    a[np.arange(n) % 16, np.arange(n) // 16] = flat.astype(np.int16)
    return np.tile(a, (8, 1))


def _s_dev(S_core):
    """[T, 128, DT] -> [128, T*DT] bf16."""
    T = S_core.shape[0]
    return np.ascontiguousarray(
        S_core.transpose(1, 0, 2).reshape(128, T * DT)).astype(BF16)


def _w_dev(W):
    """[F, fo] -> [128, KC*fo] bf16 (chunk k at cols [k*fo, (k+1)*fo))."""
    fi, fo = W.shape
    k = fi // 128
    return np.ascontiguousarray(
        W.reshape(k, 128, fo).transpose(1, 0, 2).reshape(128, k * fo)).astype(BF16)


def _prep(x, edge_index, edge_weight, W1, b1, W2, b2, Wc, bc):
    row = np.asarray(edge_index[0], np.int64)
    col = np.asarray(edge_index[1], np.int64)
    w = np.asarray(edge_weight, np.float32)

    # GCN norm (layers 1 & 2): deg over dest (col) + 1 self loop.
    deg = np.zeros(N, np.float32)
    np.add.at(deg, col, w)
    deg += 1.0
    dinv = (1.0 / np.sqrt(deg)).astype(np.float32)
    g_src = np.concatenate([row, np.arange(N)])
    g_dst = np.concatenate([col, np.arange(N)])
    g_val = np.concatenate([dinv[row] * w * dinv[col], dinv * dinv]).astype(np.float32)

    # Cheb: drop self loops, deg over src (row), negate (lhat = -A_norm).
    keep = row != col
    r0, c0, w0 = row[keep], col[keep], w[keep]
    deg2 = np.zeros(N, np.float32)
    np.add.at(deg2, r0, w0)
    dinv2 = np.where(deg2 > 0, 1.0 / np.sqrt(deg2), 0.0).astype(np.float32)
    c_val = -(dinv2[r0] * w0 * dinv2[c0]).astype(np.float32)

    (ETGA, idxga, Sga), (ETGB, idxgb, Sgb) = _build_edge_tiles_split(
        g_src, g_dst, g_val)
    (ETCA, idxca, Sca), (ETCB, idxcb, Scb) = _build_edge_tiles_split(
        r0, c0, c_val)

    x = np.asarray(x, np.float32)
    x_pad = np.zeros((NTOT, F), BF16)
    x_pad[_to_padded_id(np.arange(N))] = x.astype(BF16)

    Wc = np.asarray(Wc, np.float32)
    com = dict(
        x_bf=x_pad,
        w1=_w_dev(np.asarray(W1, np.float32)),
        w2=_w_dev(np.asarray(W2, np.float32)),
        wcat=_w_dev(np.concatenate([Wc[1], Wc[2]], axis=1)),
        wa=_w_dev(Wc[0] - Wc[2]),
        ident=np.eye(128, dtype=BF16),
    )
    biases = (np.asarray(b1, np.float32), np.asarray(b2, np.float32),
              np.asarray(bc, np.float32))
    in_maps = []
    for c in range(P):
        m = dict(com)
        m["idxga"] = _idx_dev(idxga[c])
        m["sga"] = _s_dev(Sga[c])
        m["idxgb"] = _idx_dev(idxgb[c])
        m["sgb"] = _s_dev(Sgb[c])
        m["idxca"] = _idx_dev(idxca[c])
        m["sca"] = _s_dev(Sca[c])
        m["idxcb"] = _idx_dev(idxcb[c])
        m["scb"] = _s_dev(Scb[c])
        in_maps.append(m)
    ETs = (ETGA, ETGB, ETCA, ETCB)
    return ETs, biases, in_maps


# ------------------------------------------------------------- bass program

_CACHE = {}


def _build_program(ETs, has_bias):
    import os
    key = (ETs, has_bias, os.environ.get("GNN_PHASES", "9"))
    if key in _CACHE:
        return _CACHE[key]
    ETGA, ETGB, ETCA, ETCB = ETs
    ETMAX = max(max(e) for e in ETs)

    nc = bacc.Bacc("TRN2", target_bir_lowering=False, num_devices=P,
                   num_swdge_queues=4)
    x_bf = nc.dram_tensor("x_bf", [NTOT, F], BF16D, kind="ExternalInput")
    srcs = {}
    for nm, ET in (("ga", ETGA), ("gb", ETGB), ("ca", ETCA), ("cb", ETCB)):
        T = sum(ET)
        srcs["idx" + nm] = nc.dram_tensor(
            "idx" + nm, [128, T * 8], I16, kind="ExternalInput")
        srcs["s" + nm] = nc.dram_tensor(
            "s" + nm, [128, T * DT], BF16D, kind="ExternalInput")
    w1 = nc.dram_tensor("w1", [128, KC * F], BF16D, kind="ExternalInput")
    w2 = nc.dram_tensor("w2", [128, KC * F], BF16D, kind="ExternalInput")
    wcat = nc.dram_tensor("wcat", [128, KC * F], BF16D, kind="ExternalInput")
    wa = nc.dram_tensor("wa", [128, KC * DOUT], BF16D, kind="ExternalInput")
    ident = nc.dram_tensor("ident", [128, 128], BF16D, kind="ExternalInput")
    if has_bias:
        brows = nc.dram_tensor("brows", [1, 2 * F + DOUT], FP32, kind="ExternalInput")
    outp = nc.dram_tensor("out", [NPAD, DOUT], FP32, kind="ExternalOutput")

    h1c = nc.dram_tensor("h1c", [NPAD, F], BF16D, kind="Internal")
    h1f = nc.dram_tensor("h1f", [NTOT, F], BF16D, kind="Internal", addr_space="Shared")
    y12c = nc.dram_tensor("y12c", [NPAD, F], BF16D, kind="Internal")
    y12f = nc.dram_tensor("y12f", [NTOT, F], BF16D, kind="Internal",
                          addr_space="Shared")
    z2c = nc.dram_tensor("z2c", [NPAD, DOUT], BF16D, kind="Internal")
    z2f = nc.dram_tensor("z2f", [NTOT, DOUT], BF16D, kind="Internal",
                         addr_space="Shared")

    Exp = mybir.ActivationFunctionType.Exp
    Alu = mybir.AluOpType

    offs = {nm: np.cumsum([0] + list(ET[:-1]))
            for nm, ET in (("ga", ETGA), ("gb", ETGB), ("ca", ETCA), ("cb", ETCB))}
    ETd = {"ga": ETGA, "gb": ETGB, "ca": ETCA, "cb": ETCB}

    with tile.TileContext(nc) as tc:
        with (
            tc.tile_pool(name="const", bufs=1) as cpool,
            tc.tile_pool(name="keep", bufs=1) as kpool,
            tc.tile_pool(name="msgs", bufs=3) as mpool,
            tc.tile_pool(name="work", bufs=3) as wpool,
            tc.tile_pool(name="psum", bufs=2, space="PSUM") as ppool,
            tc.tile_pool(name="psum3", bufs=3, space="PSUM") as ppool3,
        ):
            lib = nc.gpsimd.load_library(library_config.mlp)

            id_sb = cpool.tile([128, 128], BF16D, tag="id")
            nc.sync.dma_start(id_sb[:], ident[:])

            # Per-tile chunked loads of idx + S so tile-0 work starts early.
            idx_sb = {}
            s_sb = {}
            for nm in ("ga", "gb", "ca", "cb"):
                T = sum(ETd[nm])
                idx_sb[nm] = cpool.tile([128, T * 8], I16, tag="i" + nm,
                                        name="idx_" + nm)
                s_sb[nm] = cpool.tile([128, T * DT], BF16D, tag="s" + nm,
                                      name="s_" + nm)
            for t in range(NDT):
                for nm in ("ga", "gb", "ca", "cb"):
                    o, e = offs[nm][t], ETd[nm][t]
                    a, b = o * 8, (o + e) * 8
                    nc.sync.dma_start(idx_sb[nm][:, a:b], srcs["idx" + nm][:, a:b])
                    a, b = o * DT, (o + e) * DT
                    nc.sync.dma_start(s_sb[nm][:, a:b], srcs["s" + nm][:, a:b])

            w1_sb = cpool.tile([128, KC * F], BF16D, tag="w1")
            nc.sync.dma_start(w1_sb[:], w1[:])
            w2_sb = cpool.tile([128, KC * F], BF16D, tag="w2")
            nc.sync.dma_start(w2_sb[:], w2[:])
            wcat_sb = cpool.tile([128, KC * F], BF16D, tag="wcat")
            nc.sync.dma_start(wcat_sb[:], wcat[:])
            wa_sb = cpool.tile([128, KC * DOUT], BF16D, tag="wa")
            nc.sync.dma_start(wa_sb[:], wa[:])
            if has_bias:
                br_sb = cpool.tile([1, 2 * F + DOUT], FP32, tag="br")
                nc.sync.dma_start(br_sb[:], brows[:])
                ones_sb = cpool.tile([1, 128], FP32, tag="ones")
                nc.vector.memset(ones_sb[:], 1.0)

            # step-A aggregates kept across the AG boundary. One buffer,
            # reused by L2 / cheb hop 1 / cheb hop 2 (per-tile WAR deps
            # follow the pipeline order, so reuse costs no stalls).
            aggA = kpool.tile([128, NDT, F], BF16D, tag="aggA")
            aggA1 = aggA
            aggA2 = aggA[:, :, :DOUT]
            acc0k = kpool.tile([128, NDT, DOUT], BF16D, tag="acc0k")
            g1z2k = kpool.tile([128, NDT, F], BF16D, tag="g1z2k")

            first_gather = [0]
            qctr = [0]

            def scatter_into(ps, src_ap, nm, t, width, start, stop, mtag):
                """Gather half-`nm` sources of dest-tile t and accumulate
                their one-hot matmuls into psum `ps` ([128, width])."""
                o = offs[nm][t]
                et = ETd[nm][t]
                msgs = mpool.tile([128, ETMAX, width], BF16D, tag=mtag)
                isb = idx_sb[nm]
                ssb = s_sb[nm]
                nq = min(4, et)
                bounds = [et * i // nq for i in range(nq + 1)]
                for a, b in zip(bounds[:-1], bounds[1:]):
                    if b <= a:
                        continue
                    q = qctr[0] % 4
                    qctr[0] += 1
                    gi = nc.gpsimd.dma_gather(
                        msgs[:, a:b, :], src_ap,
                        isb[:, (o + a) * 8:(o + b) * 8],
                        (b - a) * 128, (b - a) * 128, width,
                        single_packet=False, queue_num=q)
                    if first_gather[0] < 4:
                        add_dep_helper(gi.ins, lib.ins,
                                       reason="mlp lib before gather")
                        first_gather[0] += 1
                for g in range(et):
                    nc.tensor.matmul(
                        ps[:, :width],
                        ssb[:, (o + g) * DT:(o + g + 1) * DT],
                        msgs[:, g, :],
                        start=(start and g == 0), stop=(stop and g == et - 1))

            def celu(z_ap, width, out_ap):
                """out = max(z,0) + min(exp(z)-1, 0)."""
                e = wpool.tile([128, F], FP32, tag="e")
                nc.scalar.activation(e[:, :width], z_ap, Exp)
                nc.vector.tensor_scalar(
                    e[:, :width], e[:, :width], 1.0, 0.0,
                    Alu.subtract, Alu.min)
                nc.vector.scalar_tensor_tensor(
                    out_ap, z_ap, 0.0, e[:, :width], Alu.max, Alu.add)

            def gemm_bias(z_ps, width, b_off):
                if has_bias:
                    nc.tensor.matmul(
                        z_ps, ones_sb[:],
                        br_sb[:, b_off:b_off + width],
                        start=False, stop=False)

            def allgather_chunk(cin, cout, j):
                nc.gpsimd.collective_compute(
                    "AllGather", Alu.bypass,
                    replica_groups=[list(range(P))],
                    ins=[cin[j * CH:(j + 1) * CH, :]],
                    outs=[cout[j * P * CH:(j + 1) * P * CH, :]])

            def transpose_kc(src_ap, dst_tile):
                """[128, F] node-major -> [128, KC, 128] feature-major."""
                tps = ppool.tile([128, KC, 128], BF16D, tag="tps")
                for k in range(KC):
                    nc.tensor.transpose(
                        tps[:, k, :], src_ap[:, k * 128:(k + 1) * 128], id_sb[:])
                nc.vector.tensor_copy(dst_tile, tps[:])

            import os
            PH = int(os.environ.get("GNN_PHASES", "9"))

            # ---- layer 1: h1 = celu((Ag @ x) @ W1 + b1); x replicated so
            # both source halves are available immediately.
            for t in range(NDT):
                ps = ppool3.tile([128, F], FP32, tag="ps")
                scatter_into(ps, x_bf[0:HALF, :], "ga", t, F, True, False, "ms")
                scatter_into(ps, x_bf[HALF:NTOT, :], "gb", t, F, False, True, "ms")
                agg = wpool.tile([128, F], BF16D, tag="agg")
                nc.vector.tensor_copy(agg[:], ps[:])
                aggT = wpool.tile([128, KC, 128], BF16D, tag="aggT")
                transpose_kc(agg, aggT[:])
                z = ppool.tile([128, F], FP32, tag="z")
                for k in range(KC):
                    nc.tensor.matmul(
                        z[:], aggT[:, k, :], w1_sb[:, k * F:(k + 1) * F],
                        start=(k == 0), stop=(k == KC - 1))
                gemm_bias(z[:], F, 0)
                h = wpool.tile([128, F], BF16D, tag="h")
                celu(z[:], F, h[:])
                nc.sync.dma_start(h1c[t * 128:(t + 1) * 128, :], h[:])
                if PH >= 2 and (t + 1) % (NDT // NCH) == 0:
                    allgather_chunk(h1c, h1f, (t + 1) // (NDT // NCH) - 1)

            # ---- layer 2 step A: aggregate half-A sources of h1
            if PH >= 3:
                for t in range(NDT):
                    ps = ppool3.tile([128, F], FP32, tag="ps")
                    scatter_into(ps, h1f[0:HALF, :], "ga", t, F, True, True, "ms")
                    nc.vector.tensor_copy(aggA[:, t, :], ps[:])

            # ---- layer 2 step B + cheb head:
            # h2 = celu((aggA+aggB) @ W2 + b2)
            # y12 = h2 @ [W1c|W2c] -> DRAM (+AG);  acc0 = h2 @ (W0c-W2c) + bc
            if PH >= 4:
                for t in range(NDT):
                    ps = ppool3.tile([128, F], FP32, tag="ps")
                    scatter_into(ps, h1f[HALF:NTOT, :], "gb", t, F, True, True, "ms")
                    agg = wpool.tile([128, F], BF16D, tag="agg")
                    nc.vector.tensor_tensor(agg[:], ps[:], aggA[:, t, :], Alu.add)
                    aggT = wpool.tile([128, KC, 128], BF16D, tag="aggT")
                    transpose_kc(agg, aggT[:])
                    z = ppool.tile([128, F], FP32, tag="z")
                    for k in range(KC):
                        nc.tensor.matmul(
                            z[:], aggT[:, k, :], w2_sb[:, k * F:(k + 1) * F],
                            start=(k == 0), stop=(k == KC - 1))
                    gemm_bias(z[:], F, F)
                    h2 = wpool.tile([128, F], BF16D, tag="h")
                    celu(z[:], F, h2[:])
                    h2T = wpool.tile([128, KC, 128], BF16D, tag="h2T")
                    transpose_kc(h2, h2T[:])
                    y12 = ppool.tile([128, F], FP32, tag="z")
                    for k in range(KC):
                        nc.tensor.matmul(
                            y12[:], h2T[:, k, :], wcat_sb[:, k * F:(k + 1) * F],
                            start=(k == 0), stop=(k == KC - 1))
                    y12s = wpool.tile([128, F], BF16D, tag="y12s")
                    nc.vector.tensor_copy(y12s[:], y12[:])
                    nc.sync.dma_start(y12c[t * 128:(t + 1) * 128, :], y12s[:])
                    acc = ppool.tile([128, F], FP32, tag="z")
                    for k in range(KC):
                        nc.tensor.matmul(
                            acc[:, :DOUT], h2T[:, k, :],
                            wa_sb[:, k * DOUT:(k + 1) * DOUT],
                            start=(k == 0), stop=(k == KC - 1))
                    gemm_bias(acc[:, :DOUT], DOUT, 2 * F)
                    nc.vector.tensor_copy(acc0k[:, t, :], acc[:, :DOUT])
                    if PH >= 5 and (t + 1) % (NDT // NCH) == 0:
                        allgather_chunk(y12c, y12f, (t + 1) // (NDT // NCH) - 1)

            # ---- cheb hop 1 step A on y12
            if PH >= 6:
                for t in range(NDT):
                    ps = ppool3.tile([128, F], FP32, tag="ps")
                    scatter_into(ps, y12f[0:HALF, :], "ca", t, F, True, True, "ms")
                    nc.vector.tensor_copy(aggA1[:, t, :], ps[:])

            # ---- cheb hop 1 step B: [g1 | z2] = lhat([y1 | y2]); z2 -> AG
            if PH >= 7:
                for t in range(NDT):
                    ps = ppool3.tile([128, F], FP32, tag="ps")
                    scatter_into(ps, y12f[HALF:NTOT, :], "cb", t, F, True, True, "ms")
                    nc.vector.tensor_tensor(
                        g1z2k[:, t, :], ps[:], aggA1[:, t, :], Alu.add)
                    nc.sync.dma_start(z2c[t * 128:(t + 1) * 128, :],
                                      g1z2k[:, t, DOUT:F])
                    if PH >= 8 and (t + 1) % (NDT // NCH) == 0:
                        allgather_chunk(z2c, z2f, (t + 1) // (NDT // NCH) - 1)

            # ---- cheb hop 2 step A on z2
            if PH >= 9:
                for t in range(NDT):
                    ps = ppool3.tile([128, F], FP32, tag="ps")
                    scatter_into(ps, z2f[0:HALF, :], "ca", t, DOUT, True, True, "ms2")
                    nc.vector.tensor_copy(aggA2[:, t, :], ps[:, :DOUT])

                # ---- cheb hop 2 step B + output:
                # out = celu(acc0 + g1 + 2*(aggA2+aggB2))
                for t in range(NDT):
                    ps = ppool3.tile([128, F], FP32, tag="ps")
                    scatter_into(ps, z2f[HALF:NTOT, :], "cb", t, DOUT, True, True,
                                 "ms2")
                    g2 = wpool.tile([128, DOUT], FP32, tag="g2")
                    nc.vector.tensor_tensor(
                        g2[:], ps[:, :DOUT], aggA2[:, t, :], Alu.add)
                    s2 = wpool.tile([128, DOUT], FP32, tag="s2")
                    nc.vector.tensor_tensor(
                        s2[:], acc0k[:, t, :], g1z2k[:, t, 0:DOUT], Alu.add)
                    zf = wpool.tile([128, DOUT], FP32, tag="zf")
                    nc.vector.scalar_tensor_tensor(
                        zf[:], g2[:], 2.0, s2[:], Alu.mult, Alu.add)
                    of = wpool.tile([128, DOUT], FP32, tag="of")
                    celu(zf[:], DOUT, of[:])
                    nc.sync.dma_start(outp[t * 128:(t + 1) * 128, :], of[:])

    nc.compile()
    _CACHE[key] = nc
    return nc


# ------------------------------------------------------------------- driver

def _run(inputs, trace=False, tmpdir=None):
    ETs, biases, in_maps = _prep(**inputs)
    has_bias = any(np.any(b != 0) for b in biases)
    if has_bias:
        brow = np.concatenate(biases).astype(np.float32)[None, :]
        for m in in_maps:
            m["brows"] = brow
    nc = _build_program(ETs, has_bias)
    res = run_bass_kernel_spmd(nc, in_maps, core_ids=list(range(P)),
                               trace=trace, tmpdir=tmpdir)
    out = np.concatenate(
        [res.results[c]["out"][:NPC] for c in range(P)], axis=0)
    return out.astype(np.float32), res


def kernel(**inputs) -> np.ndarray:
    out, _ = _run(inputs)
    return out


# revision 7
# speedup vs baseline: 1.1214x; 1.1214x over previous
"""Trainium2 Bass kernel for the ChebConv GNN problem
(nn_ChebConvConvolutional): 2x GCNConv + 1x ChebConv(K=3), N=10000 nodes,
E=160000 edges, F=512, celu activations.

Strategy (8 NeuronCores, SPMD):
  * Nodes are sharded 1250/core (padded to 1280). Edges are sharded by
    destination core and grouped into 128-dest tiles; per dest-tile the
    source nodes are deduplicated and the edge weights are baked into dense
    [128 src x 128 dst] one-hot "S" matrices (GCN self-loops folded in as
    edges with value dinv^2, Cheb normalization negated so the scatter
    directly produces lhat).
  * Pipelined AllGather: every aggregation's sources are split into two
    halves (local rows 0-639 / 640-1279 of each rank). The producing layer
    issues the AG of chunk 0 after its 5th dest tile, chunk 1 at the end;
    the consuming layer first processes all half-A source blocks (needs
    only chunk 0), then half-B, so collectives overlap compute instead of
    stalling the PE (which also avoids HAM cold-clock restarts).
  * ChebConv is computed transform-first:
        y1 = h2@W1, y2 = h2@W2, acc0 = h2@(W0-W2) + bc
        out = celu(acc0 + lhat(y1) + 2*lhat(lhat(y2)))
    so the second-hop aggregate and its AllGather are only 256 wide, and
    no node-major T_k tensors are materialized or transposed.
"""
import numpy as np
import ml_dtypes

import concourse.bacc as bacc
import concourse.mybir as mybir
import concourse.tile as tile
from concourse import library_config
from concourse.bass_utils import run_bass_kernel_spmd
from concourse.tile import add_dep_helper

BF16 = ml_dtypes.bfloat16
FP32 = mybir.dt.float32
BF16D = mybir.dt.bfloat16
I16 = mybir.dt.int16

P = 8            # cores
N = 10000        # nodes
NPC = N // P     # nodes per core
NPAD = 1280      # padded nodes per core
NTOT = NPAD * P
F = 512          # feature width of x / h1 / h2 / [y1|y2]
DOUT = 256
DT = 128         # dests per dest tile
NDT = NPAD // DT # dest tiles per core
KC = F // 128    # contraction chunks (4)
NCH = 2          # AllGather chunks per layer (source-split pipelining)
CH = NPAD // NCH # local rows per AG chunk (640)
HALF = P * CH    # global padded rows per source half (5120)


# ----------------------------------------------------------------- host prep

def _to_padded_id(n):
    """Global node id -> row in the chunked-AllGather global layout:
    [NCH chunks][P ranks][CH rows]."""
    r = n // NPC
    l = n % NPC
    j = l // CH
    return j * (P * CH) + r * CH + (l % CH)


def _build_edge_tiles_split(src, dst, val):
    """Shard by dest core, tile by 128 dests, dedup sources per tile, then
    split each tile's sources by AG half (padded id </>= HALF).
    Returns, for each half X in (A, B):
      ETX [NDT], idxX [P, TX, 128] int32 (ids relative to the half),
      SX [P, TX, 128, DT]."""
    order = np.argsort(dst, kind="stable")
    src, dst, val = src[order], dst[order], val[order]
    core_of = dst // NPC
    core_starts = np.searchsorted(core_of, np.arange(P + 1))
    per_core = []
    for c in range(P):
        lo, hi = core_starts[c], core_starts[c + 1]
        s, d, v = src[lo:hi], dst[lo:hi] - c * NPC, val[lo:hi]
        tile_of = d // DT
        tile_starts = np.searchsorted(tile_of, np.arange(NDT + 1))
        groups = []
        for t in range(NDT):
            a, b = tile_starts[t], tile_starts[t + 1]
            st, dl, vt = s[a:b], d[a:b] - t * DT, v[a:b]
            uniq, inv = np.unique(st, return_inverse=True)
            halves = []
            if len(uniq) == 0:
                halves = [(np.zeros(1, np.int64), np.zeros((1, DT), np.float32))] * 2
            else:
                S = np.zeros((len(uniq), DT), np.float32)
                np.add.at(S, (inv, dl), vt)
                pid = _to_padded_id(uniq)
                for h in range(2):
                    m = (pid < HALF) if h == 0 else (pid >= HALF)
                    if not np.any(m):
                        halves.append((np.zeros(1, np.int64),
                                       np.zeros((1, DT), np.float32)))
                    else:
                        halves.append((pid[m] - h * HALF, S[m]))
            groups.append(halves)
        per_core.append(groups)

    out = []
    for h in range(2):
        ET = [max(max((len(per_core[c][t][h][0]) + 127) // 128, 1)
                  for c in range(P)) for t in range(NDT)]
        T = sum(ET)
        off = np.cumsum([0] + ET[:-1])
        idx = np.zeros((P, T, 128), np.int32)
        S_all = np.zeros((P, T, 128, DT), np.float32)
        for c in range(P):
            for t in range(NDT):
                ids, S = per_core[c][t][h]
                n = len(ids)
                o = off[t]
                idx[c, o:o + (n + 127) // 128].reshape(-1)[:n] = ids
                S_all[c, o:o + (n + 127) // 128].reshape(-1, DT)[:n] = S
        out.append((tuple(ET), idx, S_all))
    return out


def _idx_dev(idx_core):
    """[T, 128] int32 -> [128, T*8] int16 (wrap 16 partitions, replicate x8)."""
    flat = idx_core.reshape(-1)
    n = len(flat)
    a = np.zeros((16, n // 16), np.int16)
    a[np.arange(n) % 16, np.arange(n) // 16] = flat.astype(np.int16)
    return np.tile(a, (8, 1))


def _s_dev(S_core):
    """[T, 128, DT] -> [128, T*DT] bf16."""
    T = S_core.shape[0]
    return np.ascontiguousarray(
        S_core.transpose(1, 0, 2).reshape(128, T * DT)).astype(BF16)


def _w_dev(W):
    """[F, fo] -> [128, KC*fo] bf16 (chunk k at cols [k*fo, (k+1)*fo))."""
    fi, fo = W.shape
    k = fi // 128
    return np.ascontiguousarray(
        W.reshape(k, 128, fo).transpose(1, 0, 2).reshape(128, k * fo)).astype(BF16)


def _prep(x, edge_index, edge_weight, W1, b1, W2, b2, Wc, bc):
    row = np.asarray(edge_index[0], np.int64)
    col = np.asarray(edge_index[1], np.int64)
    w = np.asarray(edge_weight, np.float32)

    # GCN norm (layers 1 & 2): deg over dest (col) + 1 self loop.
    deg = np.zeros(N, np.float32)
    np.add.at(deg, col, w)
    deg += 1.0
    dinv = (1.0 / np.sqrt(deg)).astype(np.float32)
    g_src = np.concatenate([row, np.arange(N)])
    g_dst = np.concatenate([col, np.arange(N)])
    g_val = np.concatenate([dinv[row] * w * dinv[col], dinv * dinv]).astype(np.float32)

    # Cheb: drop self loops, deg over src (row), negate (lhat = -A_norm).
    keep = row != col
    r0, c0, w0 = row[keep], col[keep], w[keep]
    deg2 = np.zeros(N, np.float32)
    np.add.at(deg2, r0, w0)
    dinv2 = np.where(deg2 > 0, 1.0 / np.sqrt(deg2), 0.0).astype(np.float32)
    c_val = -(dinv2[r0] * w0 * dinv2[c0]).astype(np.float32)

    (ETGA, idxga, Sga), (ETGB, idxgb, Sgb) = _build_edge_tiles_split(
        g_src, g_dst, g_val)
    (ETCA, idxca, Sca), (ETCB, idxcb, Scb) = _build_edge_tiles_split(
        r0, c0, c_val)

    x = np.asarray(x, np.float32)
    x_pad = np.zeros((NTOT, F), BF16)
    x_pad[_to_padded_id(np.arange(N))] = x.astype(BF16)

    Wc = np.asarray(Wc, np.float32)
    com = dict(
        x_bf=x_pad,
        w1=_w_dev(np.asarray(W1, np.float32)),
        w2=_w_dev(np.asarray(W2, np.float32)),
        wcat=_w_dev(np.concatenate([Wc[1], Wc[2]], axis=1)),
        wa=_w_dev(Wc[0] - Wc[2]),
        ident=np.eye(128, dtype=BF16),
    )
    biases = (np.asarray(b1, np.float32), np.asarray(b2, np.float32),
              np.asarray(bc, np.float32))
    in_maps = []
    for c in range(P):
        m = dict(com)
        m["idxga"] = _idx_dev(idxga[c])
        m["sga"] = _s_dev(Sga[c])
        m["idxgb"] = _idx_dev(idxgb[c])
        m["sgb"] = _s_dev(Sgb[c])
        m["idxca"] = _idx_dev(idxca[c])
        m["sca"] = _s_dev(Sca[c])
        m["idxcb"] = _idx_dev(idxcb[c])
        m["scb"] = _s_dev(Scb[c])
        in_maps.append(m)
    ETs = (ETGA, ETGB, ETCA, ETCB)
    return ETs, biases, in_maps


# ------------------------------------------------------------- bass program

_CACHE = {}


def _build_program(ETs, has_bias):
    import os
    key = (ETs, has_bias, os.environ.get("GNN_PHASES", "9"))
    if key in _CACHE:
        return _CACHE[key]
    ETGA, ETGB, ETCA, ETCB = ETs
    ETMAX = max(max(e) for e in ETs)

    nc = bacc.Bacc("TRN2", target_bir_lowering=False, num_devices=P,
                   num_swdge_queues=4)
    x_bf = nc.dram_tensor("x_bf", [NTOT, F], BF16D, kind="ExternalInput")
    srcs = {}
    for nm, ET in (("ga", ETGA), ("gb", ETGB), ("ca", ETCA), ("cb", ETCB)):
        T = sum(ET)
        srcs["idx" + nm] = nc.dram_tensor(
            "idx" + nm, [128, T * 8], I16, kind="ExternalInput")
        srcs["s" + nm] = nc.dram_tensor(
            "s" + nm, [128, T * DT], BF16D, kind="ExternalInput")
    w1 = nc.dram_tensor("w1", [128, KC * F], BF16D, kind="ExternalInput")
    w2 = nc.dram_tensor("w2", [128, KC * F], BF16D, kind="ExternalInput")
    wcat = nc.dram_tensor("wcat", [128, KC * F], BF16D, kind="ExternalInput")
    wa = nc.dram_tensor("wa", [128, KC * DOUT], BF16D, kind="ExternalInput")
    ident = nc.dram_tensor("ident", [128, 128], BF16D, kind="ExternalInput")
    if has_bias:
        brows = nc.dram_tensor("brows", [1, 2 * F + DOUT], FP32, kind="ExternalInput")
    outp = nc.dram_tensor("out", [NPAD, DOUT], FP32, kind="ExternalOutput")

    warm_i = nc.dram_tensor("warm_i", [1, 128], BF16D, kind="Internal")
    warm_o = nc.dram_tensor("warm_o", [P, 128], BF16D, kind="Internal",
                            addr_space="Shared")
    h1c = nc.dram_tensor("h1c", [NPAD, F], BF16D, kind="Internal")
    h1f = nc.dram_tensor("h1f", [NTOT, F], BF16D, kind="Internal", addr_space="Shared")
    y12c = nc.dram_tensor("y12c", [NPAD, F], BF16D, kind="Internal")
    y12f = nc.dram_tensor("y12f", [NTOT, F], BF16D, kind="Internal",
                          addr_space="Shared")
    z2c = nc.dram_tensor("z2c", [NPAD, DOUT], BF16D, kind="Internal")
    z2f = nc.dram_tensor("z2f", [NTOT, DOUT], BF16D, kind="Internal",
                         addr_space="Shared")

    Exp = mybir.ActivationFunctionType.Exp
    Alu = mybir.AluOpType

    offs = {nm: np.cumsum([0] + list(ET[:-1]))
            for nm, ET in (("ga", ETGA), ("gb", ETGB), ("ca", ETCA), ("cb", ETCB))}
    ETd = {"ga": ETGA, "gb": ETGB, "ca": ETCA, "cb": ETCB}

    with tile.TileContext(nc) as tc:
        with (
            tc.tile_pool(name="const", bufs=1) as cpool,
            tc.tile_pool(name="keep", bufs=1) as kpool,
            tc.tile_pool(name="msgs", bufs=3) as mpool,
            tc.tile_pool(name="work", bufs=3) as wpool,
            tc.tile_pool(name="psum", bufs=2, space="PSUM") as ppool,
            tc.tile_pool(name="psum3", bufs=3, space="PSUM") as ppool3,
        ):
            lib = nc.gpsimd.load_library(library_config.mlp)

            # Tiny warm-up collective issued first: absorbs the one-time CC
            # init barrier into the load window so the first real AllGather
            # starts at its issue point instead of queueing behind it.
            nc.gpsimd.collective_compute(
                "AllGather", mybir.AluOpType.bypass,
                replica_groups=[list(range(P))],
                ins=[warm_i[0:1, :]], outs=[warm_o[0:P, :]])

            id_sb = cpool.tile([128, 128], BF16D, tag="id")
            nc.sync.dma_start(id_sb[:], ident[:])

            # Per-tile chunked loads of idx + S so tile-0 work starts early.
            idx_sb = {}
            s_sb = {}
            for nm in ("ga", "gb", "ca", "cb"):
                T = sum(ETd[nm])
                idx_sb[nm] = cpool.tile([128, T * 8], I16, tag="i" + nm,
                                        name="idx_" + nm)
                s_sb[nm] = cpool.tile([128, T * DT], BF16D, tag="s" + nm,
                                      name="s_" + nm)
            for t in range(NDT):
                for nm in ("ga", "gb", "ca", "cb"):
                    o, e = offs[nm][t], ETd[nm][t]
                    a, b = o * 8, (o + e) * 8
                    nc.sync.dma_start(idx_sb[nm][:, a:b], srcs["idx" + nm][:, a:b])
                    a, b = o * DT, (o + e) * DT
                    nc.sync.dma_start(s_sb[nm][:, a:b], srcs["s" + nm][:, a:b])

            w1_sb = cpool.tile([128, KC * F], BF16D, tag="w1")
            nc.sync.dma_start(w1_sb[:], w1[:])
            w2_sb = cpool.tile([128, KC * F], BF16D, tag="w2")
            nc.sync.dma_start(w2_sb[:], w2[:])
            wcat_sb = cpool.tile([128, KC * F], BF16D, tag="wcat")
            nc.sync.dma_start(wcat_sb[:], wcat[:])
            wa_sb = cpool.tile([128, KC * DOUT], BF16D, tag="wa")
            nc.sync.dma_start(wa_sb[:], wa[:])
            if has_bias:
                br_sb = cpool.tile([1, 2 * F + DOUT], FP32, tag="br")
                nc.sync.dma_start(br_sb[:], brows[:])
                ones_sb = cpool.tile([1, 128], FP32, tag="ones")
                nc.vector.memset(ones_sb[:], 1.0)

            # step-A aggregates kept across the AG boundary. One buffer,
            # reused by L2 / cheb hop 1 / cheb hop 2 (per-tile WAR deps
            # follow the pipeline order, so reuse costs no stalls).
            aggA = kpool.tile([128, NDT, F], BF16D, tag="aggA")
            aggA1 = aggA
            aggA2 = aggA[:, :, :DOUT]
            acc0k = kpool.tile([128, NDT, DOUT], BF16D, tag="acc0k")
            g1z2k = kpool.tile([128, NDT, F], BF16D, tag="g1z2k")

            first_gather = [0]
            qctr = [0]

            def scatter_into(ps, src_ap, nm, t, width, start, stop, mtag):
                """Gather half-`nm` sources of dest-tile t and accumulate
                their one-hot matmuls into psum `ps` ([128, width])."""
                o = offs[nm][t]
                et = ETd[nm][t]
                msgs = mpool.tile([128, ETMAX, width], BF16D, tag=mtag)
                isb = idx_sb[nm]
                ssb = s_sb[nm]
                nq = min(4, et)
                bounds = [et * i // nq for i in range(nq + 1)]
                for a, b in zip(bounds[:-1], bounds[1:]):
                    if b <= a:
                        continue
                    q = qctr[0] % 4
                    qctr[0] += 1
                    gi = nc.gpsimd.dma_gather(
                        msgs[:, a:b, :], src_ap,
                        isb[:, (o + a) * 8:(o + b) * 8],
                        (b - a) * 128, (b - a) * 128, width,
                        single_packet=False, queue_num=q)
                    if first_gather[0] < 4:
                        add_dep_helper(gi.ins, lib.ins,
                                       reason="mlp lib before gather")
                        first_gather[0] += 1
                for g in range(et):
                    nc.tensor.matmul(
                        ps[:, :width],
                        ssb[:, (o + g) * DT:(o + g + 1) * DT],
                        msgs[:, g, :],
                        start=(start and g == 0), stop=(stop and g == et - 1))

            def celu(z_ap, width, out_ap):
                """out = max(z,0) + min(exp(z)-1, 0)."""
                e = wpool.tile([128, F], FP32, tag="e")
                nc.scalar.activation(e[:, :width], z_ap, Exp)
                nc.vector.tensor_scalar(
                    e[:, :width], e[:, :width], 1.0, 0.0,
                    Alu.subtract, Alu.min)
                nc.vector.scalar_tensor_tensor(
                    out_ap, z_ap, 0.0, e[:, :width], Alu.max, Alu.add)

            def gemm_bias(z_ps, width, b_off):
                if has_bias:
                    nc.tensor.matmul(
                        z_ps, ones_sb[:],
                        br_sb[:, b_off:b_off + width],
                        start=False, stop=False)

            def allgather_chunk(cin, cout, j):
                nc.gpsimd.collective_compute(
                    "AllGather", Alu.bypass,
                    replica_groups=[list(range(P))],
                    ins=[cin[j * CH:(j + 1) * CH, :]],
                    outs=[cout[j * P * CH:(j + 1) * P * CH, :]])

            def transpose_kc(src_ap, dst_tile):
                """[128, F] node-major -> [128, KC, 128] feature-major."""
                tps = ppool.tile([128, KC, 128], BF16D, tag="tps")
                for k in range(KC):
                    nc.tensor.transpose(
                        tps[:, k, :], src_ap[:, k * 128:(k + 1) * 128], id_sb[:])
                nc.vector.tensor_copy(dst_tile, tps[:])

            import os
            PH = int(os.environ.get("GNN_PHASES", "9"))

            # ---- layer 1: h1 = celu((Ag @ x) @ W1 + b1); x replicated so
            # both source halves are available immediately.
            for t in range(NDT):
                ps = ppool3.tile([128, F], FP32, tag="ps")
                scatter_into(ps, x_bf[0:HALF, :], "ga", t, F, True, False, "ms")
                scatter_into(ps, x_bf[HALF:NTOT, :], "gb", t, F, False, True, "ms")
                agg = wpool.tile([128, F], BF16D, tag="agg")
                nc.vector.tensor_copy(agg[:], ps[:])
                aggT = wpool.tile([128, KC, 128], BF16D, tag="aggT")
                transpose_kc(agg, aggT[:])
                z = ppool.tile([128, F], FP32, tag="z")
                for k in range(KC):
                    nc.tensor.matmul(
                        z[:], aggT[:, k, :], w1_sb[:, k * F:(k + 1) * F],
                        start=(k == 0), stop=(k == KC - 1))
                gemm_bias(z[:], F, 0)
                h = wpool.tile([128, F], BF16D, tag="h")
                celu(z[:], F, h[:])
                nc.sync.dma_start(h1c[t * 128:(t + 1) * 128, :], h[:])
                if PH >= 2 and (t + 1) % (NDT // NCH) == 0:
                    allgather_chunk(h1c, h1f, (t + 1) // (NDT // NCH) - 1)

            # ---- layer 2 step A: aggregate half-A sources of h1
            if PH >= 3:
                for t in range(NDT):
                    ps = ppool3.tile([128, F], FP32, tag="ps")
                    scatter_into(ps, h1f[0:HALF, :], "ga", t, F, True, True, "ms")
                    nc.vector.tensor_copy(aggA[:, t, :], ps[:])

            # ---- layer 2 step B + cheb head:
            # h2 = celu((aggA+aggB) @ W2 + b2)
            # y12 = h2 @ [W1c|W2c] -> DRAM (+AG);  acc0 = h2 @ (W0c-W2c) + bc
            if PH >= 4:
                for t in range(NDT):
                    ps = ppool3.tile([128, F], FP32, tag="ps")
                    scatter_into(ps, h1f[HALF:NTOT, :], "gb", t, F, True, True, "ms")
                    agg = wpool.tile([128, F], BF16D, tag="agg")
                    nc.vector.tensor_tensor(agg[:], ps[:], aggA[:, t, :], Alu.add)
                    aggT = wpool.tile([128, KC, 128], BF16D, tag="aggT")
                    transpose_kc(agg, aggT[:])
                    z = ppool.tile([128, F], FP32, tag="z")
                    for k in range(KC):
                        nc.tensor.matmul(
                            z[:], aggT[:, k, :], w2_sb[:, k * F:(k + 1) * F],
                            start=(k == 0), stop=(k == KC - 1))
                    gemm_bias(z[:], F, F)
                    h2 = wpool.tile([128, F], BF16D, tag="h")
                    celu(z[:], F, h2[:])
                    h2T = wpool.tile([128, KC, 128], BF16D, tag="h2T")
                    transpose_kc(h2, h2T[:])
                    y12 = ppool.tile([128, F], FP32, tag="z")
                    for k in range(KC):
                        nc.tensor.matmul(
                            y12[:], h2T[:, k, :], wcat_sb[:, k * F:(k + 1) * F],
                            start=(k == 0), stop=(k == KC - 1))
                    y12s = wpool.tile([128, F], BF16D, tag="y12s")
                    nc.vector.tensor_copy(y12s[:], y12[:])
                    nc.sync.dma_start(y12c[t * 128:(t + 1) * 128, :], y12s[:])
                    acc = ppool.tile([128, F], FP32, tag="z")
                    for k in range(KC):
                        nc.tensor.matmul(
                            acc[:, :DOUT], h2T[:, k, :],
                            wa_sb[:, k * DOUT:(k + 1) * DOUT],
                            start=(k == 0), stop=(k == KC - 1))
                    gemm_bias(acc[:, :DOUT], DOUT, 2 * F)
                    nc.vector.tensor_copy(acc0k[:, t, :], acc[:, :DOUT])
                    if PH >= 5 and (t + 1) % (NDT // NCH) == 0:
                        allgather_chunk(y12c, y12f, (t + 1) // (NDT // NCH) - 1)

            # ---- cheb hop 1 step A on y12
            if PH >= 6:
                for t in range(NDT):
                    ps = ppool3.tile([128, F], FP32, tag="ps")
                    scatter_into(ps, y12f[0:HALF, :], "ca", t, F, True, True, "ms")
                    nc.vector.tensor_copy(aggA1[:, t, :], ps[:])

            # ---- cheb hop 1 step B: [g1 | z2] = lhat([y1 | y2]); z2 -> AG
            if PH >= 7:
                for t in range(NDT):
                    ps = ppool3.tile([128, F], FP32, tag="ps")
                    scatter_into(ps, y12f[HALF:NTOT, :], "cb", t, F, True, True, "ms")
                    nc.vector.tensor_tensor(
                        g1z2k[:, t, :], ps[:], aggA1[:, t, :], Alu.add)
                    nc.sync.dma_start(z2c[t * 128:(t + 1) * 128, :],
                                      g1z2k[:, t, DOUT:F])
                    if PH >= 8 and (t + 1) % (NDT // NCH) == 0:
                        allgather_chunk(z2c, z2f, (t + 1) // (NDT // NCH) - 1)

            # ---- cheb hop 2 step A on z2
            if PH >= 9:
                for t in range(NDT):
                    ps = ppool3.tile([128, F], FP32, tag="ps")
                    scatter_into(ps, z2f[0:HALF, :], "ca", t, DOUT, True, True, "ms2")
                    nc.vector.tensor_copy(aggA2[:, t, :], ps[:, :DOUT])

                # ---- cheb hop 2 step B + output:
                # out = celu(acc0 + g1 + 2*(aggA2+aggB2))
                for t in range(NDT):
                    ps = ppool3.tile([128, F], FP32, tag="ps")
                    scatter_into(ps, z2f[HALF:NTOT, :], "cb", t, DOUT, True, True,
                                 "ms2")
                    g2 = wpool.tile([128, DOUT], FP32, tag="g2")
                    nc.vector.tensor_tensor(
                        g2[:], ps[:, :DOUT], aggA2[:, t, :], Alu.add)
                    s2 = wpool.tile([128, DOUT], FP32, tag="s2")
                    nc.vector.tensor_tensor(
                        s2[:], acc0k[:, t, :], g1z2k[:, t, 0:DOUT], Alu.add)
                    zf = wpool.tile([128, DOUT], FP32, tag="zf")
                    nc.vector.scalar_tensor_tensor(
                        zf[:], g2[:], 2.0, s2[:], Alu.mult, Alu.add)
                    of = wpool.tile([128, DOUT], FP32, tag="of")
                    celu(zf[:], DOUT, of[:])
                    nc.sync.dma_start(outp[t * 128:(t + 1) * 128, :], of[:])

    nc.compile()
    _CACHE[key] = nc
    return nc


# ------------------------------------------------------------------- driver

def _run(inputs, trace=False, tmpdir=None):
    ETs, biases, in_maps = _prep(**inputs)
    has_bias = any(np.any(b != 0) for b in biases)
    if has_bias:
        brow = np.concatenate(biases).astype(np.float32)[None, :]
        for m in in_maps:
            m["brows"] = brow
    nc = _build_program(ETs, has_bias)
    res = run_bass_kernel_spmd(nc, in_maps, core_ids=list(range(P)),
                               trace=trace, tmpdir=tmpdir)
    out = np.concatenate(
        [res.results[c]["out"][:NPC] for c in range(P)], axis=0)
    return out.astype(np.float32), res


def kernel(**inputs) -> np.ndarray:
    out, _ = _run(inputs)
    return out


# revision 9
# speedup vs baseline: 1.1499x; 1.0254x over previous
"""Trainium2 Bass kernel for the ChebConv GNN problem
(nn_ChebConvConvolutional): 2x GCNConv + 1x ChebConv(K=3), N=10000 nodes,
E=160000 edges, F=512, celu activations.

Strategy (8 NeuronCores, SPMD):
  * Nodes are sharded 1250/core (padded to 1280). Edges are sharded by
    destination core and grouped into 128-dest tiles; per dest-tile the
    source nodes are deduplicated and the edge weights are baked into dense
    [128 src x 128 dst] one-hot "S" matrices (GCN self-loops folded in as
    edges with value dinv^2, Cheb normalization negated so the scatter
    directly produces lhat).
  * Pipelined AllGather: every aggregation's sources are split into two
    halves (local rows 0-639 / 640-1279 of each rank). The producing layer
    issues the AG of chunk 0 after its 5th dest tile, chunk 1 at the end;
    the consuming layer first processes all half-A source blocks (needs
    only chunk 0), then half-B, so collectives overlap compute instead of
    stalling the PE (which also avoids HAM cold-clock restarts).
  * ChebConv is computed transform-first:
        y1 = h2@W1, y2 = h2@W2, acc0 = h2@(W0-W2) + bc
        out = celu(acc0 + lhat(y1) + 2*lhat(lhat(y2)))
    so the second-hop aggregate and its AllGather are only 256 wide, and
    no node-major T_k tensors are materialized or transposed.
"""
import numpy as np
import ml_dtypes

import concourse.bacc as bacc
import concourse.mybir as mybir
import concourse.tile as tile
from concourse import library_config
from concourse.bass_utils import run_bass_kernel_spmd
from concourse.tile import add_dep_helper

BF16 = ml_dtypes.bfloat16
FP32 = mybir.dt.float32
BF16D = mybir.dt.bfloat16
I16 = mybir.dt.int16

P = 8            # cores
N = 10000        # nodes
NPC = N // P     # nodes per core
NPAD = 1280      # padded nodes per core
NTOT = NPAD * P
F = 512          # feature width of x / h1 / h2 / [y1|y2]
DOUT = 256
DT = 128         # dests per dest tile
NDT = NPAD // DT # dest tiles per core
KC = F // 128    # contraction chunks (4)
NCH = 2          # AllGather chunks per layer (source-split pipelining)
CH = NPAD // NCH # local rows per AG chunk (640)
HALF = P * CH    # global padded rows per source half (5120)


# ----------------------------------------------------------------- host prep

def _to_padded_id(n):
    """Global node id -> row in the chunked-AllGather global layout:
    [NCH chunks][P ranks][CH rows]."""
    r = n // NPC
    l = n % NPC
    j = l // CH
    return j * (P * CH) + r * CH + (l % CH)


def _build_edge_tiles_split(src, dst, val):
    """Shard by dest core, tile by 128 dests, dedup sources per tile, then
    split each tile's sources by AG half (padded id </>= HALF).
    Returns, for each half X in (A, B):
      ETX [NDT], idxX [P, TX, 128] int32 (ids relative to the half),
      SX [P, TX, 128, DT]."""
    order = np.argsort(dst, kind="stable")
    src, dst, val = src[order], dst[order], val[order]
    core_of = dst // NPC
    core_starts = np.searchsorted(core_of, np.arange(P + 1))
    per_core = []
    for c in range(P):
        lo, hi = core_starts[c], core_starts[c + 1]
        s, d, v = src[lo:hi], dst[lo:hi] - c * NPC, val[lo:hi]
        tile_of = d // DT
        tile_starts = np.searchsorted(tile_of, np.arange(NDT + 1))
        groups = []
        for t in range(NDT):
            a, b = tile_starts[t], tile_starts[t + 1]
            st, dl, vt = s[a:b], d[a:b] - t * DT, v[a:b]
            uniq, inv = np.unique(st, return_inverse=True)
            halves = []
            if len(uniq) == 0:
                halves = [(np.zeros(1, np.int64), np.zeros((1, DT), np.float32))] * 2
            else:
                S = np.zeros((len(uniq), DT), np.float32)
                np.add.at(S, (inv, dl), vt)
                pid = _to_padded_id(uniq)
                for h in range(2):
                    m = (pid < HALF) if h == 0 else (pid >= HALF)
                    if not np.any(m):
                        halves.append((np.zeros(1, np.int64),
                                       np.zeros((1, DT), np.float32)))
                    else:
                        halves.append((pid[m] - h * HALF, S[m]))
            groups.append(halves)
        per_core.append(groups)

    out = []
    for h in range(2):
        ET = [max(max((len(per_core[c][t][h][0]) + 127) // 128, 1)
                  for c in range(P)) for t in range(NDT)]
        T = sum(ET)
        off = np.cumsum([0] + ET[:-1])
        idx = np.zeros((P, T, 128), np.int32)
        S_all = np.zeros((P, T, 128, DT), np.float32)
        for c in range(P):
            for t in range(NDT):
                ids, S = per_core[c][t][h]
                n = len(ids)
                o = off[t]
                idx[c, o:o + (n + 127) // 128].reshape(-1)[:n] = ids
                S_all[c, o:o + (n + 127) // 128].reshape(-1, DT)[:n] = S
        out.append((tuple(ET), idx, S_all))
    return out


def _idx_dev(idx_core):
    """[T, 128] int32 -> [128, T*8] int16 (wrap 16 partitions, replicate x8)."""
    flat = idx_core.reshape(-1)
    n = len(flat)
    a = np.zeros((16, n // 16), np.int16)
    a[np.arange(n) % 16, np.arange(n) // 16] = flat.astype(np.int16)
    return np.tile(a, (8, 1))


def _s_dev(S_core):
    """[T, 128, DT] -> [128, T*DT] bf16."""
    T = S_core.shape[0]
    return np.ascontiguousarray(
        S_core.transpose(1, 0, 2).reshape(128, T * DT)).astype(BF16)


def _w_dev(W):
    """[F, fo] -> [128, KC*fo] bf16 (chunk k at cols [k*fo, (k+1)*fo))."""
    fi, fo = W.shape
    k = fi // 128
    return np.ascontiguousarray(
        W.reshape(k, 128, fo).transpose(1, 0, 2).reshape(128, k * fo)).astype(BF16)


def _prep(x, edge_index, edge_weight, W1, b1, W2, b2, Wc, bc):
    row = np.asarray(edge_index[0], np.int64)
    col = np.asarray(edge_index[1], np.int64)
    w = np.asarray(edge_weight, np.float32)

    # GCN norm (layers 1 & 2): deg over dest (col) + 1 self loop.
    deg = np.zeros(N, np.float32)
    np.add.at(deg, col, w)
    deg += 1.0
    dinv = (1.0 / np.sqrt(deg)).astype(np.float32)
    g_src = np.concatenate([row, np.arange(N)])
    g_dst = np.concatenate([col, np.arange(N)])
    g_val = np.concatenate([dinv[row] * w * dinv[col], dinv * dinv]).astype(np.float32)

    # Cheb: drop self loops, deg over src (row), negate (lhat = -A_norm).
    keep = row != col
    r0, c0, w0 = row[keep], col[keep], w[keep]
    deg2 = np.zeros(N, np.float32)
    np.add.at(deg2, r0, w0)
    dinv2 = np.where(deg2 > 0, 1.0 / np.sqrt(deg2), 0.0).astype(np.float32)
    c_val = -(dinv2[r0] * w0 * dinv2[c0]).astype(np.float32)

    (ETGA, idxga, Sga), (ETGB, idxgb, Sgb) = _build_edge_tiles_split(
        g_src, g_dst, g_val)
    (ETCA, idxca, Sca), (ETCB, idxcb, Scb) = _build_edge_tiles_split(
        r0, c0, c_val)

    x = np.asarray(x, np.float32)
    x_pad = np.zeros((NTOT, F), BF16)
    x_pad[_to_padded_id(np.arange(N))] = x.astype(BF16)

    Wc = np.asarray(Wc, np.float32)
    com = dict(
        x_bf=x_pad,
        w1=_w_dev(np.asarray(W1, np.float32)),
        w2=_w_dev(np.asarray(W2, np.float32)),
        wcat=_w_dev(np.concatenate([Wc[1], Wc[2]], axis=1)),
        wa=_w_dev(Wc[0] - Wc[2]),
        ident=np.eye(128, dtype=BF16),
    )
    biases = (np.asarray(b1, np.float32), np.asarray(b2, np.float32),
              np.asarray(bc, np.float32))
    in_maps = []
    for c in range(P):
        m = dict(com)
        m["idxga"] = _idx_dev(idxga[c])
        m["sga"] = _s_dev(Sga[c])
        m["idxgb"] = _idx_dev(idxgb[c])
        m["sgb"] = _s_dev(Sgb[c])
        m["idxca"] = _idx_dev(idxca[c])
        m["sca"] = _s_dev(Sca[c])
        m["idxcb"] = _idx_dev(idxcb[c])
        m["scb"] = _s_dev(Scb[c])
        in_maps.append(m)
    ETs = (ETGA, ETGB, ETCA, ETCB)
    return ETs, biases, in_maps


# ------------------------------------------------------------- bass program

_CACHE = {}


def _build_program(ETs, has_bias):
    import os
    key = (ETs, has_bias, os.environ.get("GNN_PHASES", "9"))
    if key in _CACHE:
        return _CACHE[key]
    ETGA, ETGB, ETCA, ETCB = ETs
    ETMAX = max(max(e) for e in ETs)

    nc = bacc.Bacc("TRN2", target_bir_lowering=False, num_devices=P,
                   num_swdge_queues=4)
    x_bf = nc.dram_tensor("x_bf", [NTOT, F], BF16D, kind="ExternalInput")
    srcs = {}
    for nm, ET in (("ga", ETGA), ("gb", ETGB), ("ca", ETCA), ("cb", ETCB)):
        T = sum(ET)
        srcs["idx" + nm] = nc.dram_tensor(
            "idx" + nm, [128, T * 8], I16, kind="ExternalInput")
        srcs["s" + nm] = nc.dram_tensor(
            "s" + nm, [128, T * DT], BF16D, kind="ExternalInput")
    w1 = nc.dram_tensor("w1", [128, KC * F], BF16D, kind="ExternalInput")
    w2 = nc.dram_tensor("w2", [128, KC * F], BF16D, kind="ExternalInput")
    wcat = nc.dram_tensor("wcat", [128, KC * F], BF16D, kind="ExternalInput")
    wa = nc.dram_tensor("wa", [128, KC * DOUT], BF16D, kind="ExternalInput")
    ident = nc.dram_tensor("ident", [128, 128], BF16D, kind="ExternalInput")
    if has_bias:
        brows = nc.dram_tensor("brows", [1, 2 * F + DOUT], FP32, kind="ExternalInput")
    outp = nc.dram_tensor("out", [NPAD, DOUT], FP32, kind="ExternalOutput")

    warm_i = nc.dram_tensor("warm_i", [1, 128], BF16D, kind="Internal")
    warm_o = nc.dram_tensor("warm_o", [P, 128], BF16D, kind="Internal",
                            addr_space="Shared")
    h1c = nc.dram_tensor("h1c", [NPAD, F], BF16D, kind="Internal")
    h1f = nc.dram_tensor("h1f", [NTOT, F], BF16D, kind="Internal", addr_space="Shared")
    y12c = nc.dram_tensor("y12c", [NPAD, F], BF16D, kind="Internal")
    y12f = nc.dram_tensor("y12f", [NTOT, F], BF16D, kind="Internal",
                          addr_space="Shared")
    z2c = nc.dram_tensor("z2c", [NPAD, DOUT], BF16D, kind="Internal")
    z2f = nc.dram_tensor("z2f", [NTOT, DOUT], BF16D, kind="Internal",
                         addr_space="Shared")

    Exp = mybir.ActivationFunctionType.Exp
    Alu = mybir.AluOpType

    offs = {nm: np.cumsum([0] + list(ET[:-1]))
            for nm, ET in (("ga", ETGA), ("gb", ETGB), ("ca", ETCA), ("cb", ETCB))}
    ETd = {"ga": ETGA, "gb": ETGB, "ca": ETCA, "cb": ETCB}

    with tile.TileContext(nc) as tc:
        with (
            tc.tile_pool(name="const", bufs=1) as cpool,
            tc.tile_pool(name="keep", bufs=1) as kpool,
            tc.tile_pool(name="msgs", bufs=3) as mpool,
            tc.tile_pool(name="work", bufs=3) as wpool,
            tc.tile_pool(name="psum", bufs=2, space="PSUM") as ppool,
            tc.tile_pool(name="psumz", bufs=3, space="PSUM") as ppoolz,
            tc.tile_pool(name="psum3", bufs=3, space="PSUM") as ppool3,
        ):
            lib = nc.gpsimd.load_library(library_config.mlp)

            # Tiny warm-up collective issued first: absorbs the one-time CC
            # init barrier into the load window so the first real AllGather
            # starts at its issue point instead of queueing behind it.
            nc.gpsimd.collective_compute(
                "AllGather", mybir.AluOpType.bypass,
                replica_groups=[list(range(P))],
                ins=[warm_i[0:1, :]], outs=[warm_o[0:P, :]])

            id_sb = cpool.tile([128, 128], BF16D, tag="id")
            nc.sync.dma_start(id_sb[:], ident[:])

            # Per-tile chunked loads of idx + S so tile-0 work starts early.
            idx_sb = {}
            s_sb = {}
            for nm in ("ga", "gb", "ca", "cb"):
                T = sum(ETd[nm])
                idx_sb[nm] = cpool.tile([128, T * 8], I16, tag="i" + nm,
                                        name="idx_" + nm)
                s_sb[nm] = cpool.tile([128, T * DT], BF16D, tag="s" + nm,
                                      name="s_" + nm)
            for t in range(NDT):
                for nm in ("ga", "gb", "ca", "cb"):
                    o, e = offs[nm][t], ETd[nm][t]
                    a, b = o * 8, (o + e) * 8
                    nc.sync.dma_start(idx_sb[nm][:, a:b], srcs["idx" + nm][:, a:b])
                    a, b = o * DT, (o + e) * DT
                    nc.sync.dma_start(s_sb[nm][:, a:b], srcs["s" + nm][:, a:b])

            w1_sb = cpool.tile([128, KC * F], BF16D, tag="w1")
            nc.sync.dma_start(w1_sb[:], w1[:])
            w2_sb = cpool.tile([128, KC * F], BF16D, tag="w2")
            nc.sync.dma_start(w2_sb[:], w2[:])
            wcat_sb = cpool.tile([128, KC * F], BF16D, tag="wcat")
            nc.sync.dma_start(wcat_sb[:], wcat[:])
            wa_sb = cpool.tile([128, KC * DOUT], BF16D, tag="wa")
            nc.sync.dma_start(wa_sb[:], wa[:])
            if has_bias:
                br_sb = cpool.tile([1, 2 * F + DOUT], FP32, tag="br")
                nc.sync.dma_start(br_sb[:], brows[:])
                ones_sb = cpool.tile([1, 128], FP32, tag="ones")
                nc.vector.memset(ones_sb[:], 1.0)

            # step-A aggregates kept across the AG boundary. One buffer,
            # reused by L2 / cheb hop 1 / cheb hop 2 (per-tile WAR deps
            # follow the pipeline order, so reuse costs no stalls).
            aggA = kpool.tile([128, NDT, F], BF16D, tag="aggA")
            aggA1 = aggA
            aggA2 = aggA[:, :, :DOUT]
            acc0k = kpool.tile([128, NDT, DOUT], BF16D, tag="acc0k")
            g1z2k = kpool.tile([128, NDT, F], BF16D, tag="g1z2k")

            first_gather = [0]
            qctr = [0]

            def scatter_into(ps, src_ap, nm, t, width, start, stop, mtag):
                """Gather half-`nm` sources of dest-tile t and accumulate
                their one-hot matmuls into psum `ps` ([128, width])."""
                o = offs[nm][t]
                et = ETd[nm][t]
                msgs = mpool.tile([128, ETMAX, width], BF16D, tag=mtag)
                isb = idx_sb[nm]
                ssb = s_sb[nm]
                nq = min(4, et)
                bounds = [et * i // nq for i in range(nq + 1)]
                for a, b in zip(bounds[:-1], bounds[1:]):
                    if b <= a:
                        continue
                    q = qctr[0] % 4
                    qctr[0] += 1
                    gi = nc.gpsimd.dma_gather(
                        msgs[:, a:b, :], src_ap,
                        isb[:, (o + a) * 8:(o + b) * 8],
                        (b - a) * 128, (b - a) * 128, width,
                        single_packet=False, queue_num=q)
                    if first_gather[0] < 4:
                        add_dep_helper(gi.ins, lib.ins,
                                       reason="mlp lib before gather")
                        first_gather[0] += 1
                for g in range(et):
                    nc.tensor.matmul(
                        ps[:, :width],
                        ssb[:, (o + g) * DT:(o + g + 1) * DT],
                        msgs[:, g, :],
                        start=(start and g == 0), stop=(stop and g == et - 1))

            def celu(z_ap, width, out_ap):
                """out = max(z,0) + min(exp(z)-1, 0)."""
                e = wpool.tile([128, F], FP32, tag="e")
                nc.scalar.activation(e[:, :width], z_ap, Exp)
                nc.vector.tensor_scalar(
                    e[:, :width], e[:, :width], 1.0, 0.0,
                    Alu.subtract, Alu.min)
                nc.vector.scalar_tensor_tensor(
                    out_ap, z_ap, 0.0, e[:, :width], Alu.max, Alu.add)

            def gemm_bias(z_ps, width, b_off):
                if has_bias:
                    nc.tensor.matmul(
                        z_ps, ones_sb[:],
                        br_sb[:, b_off:b_off + width],
                        start=False, stop=False)

            def allgather_chunk(cin, cout, j):
                nc.gpsimd.collective_compute(
                    "AllGather", Alu.bypass,
                    replica_groups=[list(range(P))],
                    ins=[cin[j * CH:(j + 1) * CH, :]],
                    outs=[cout[j * P * CH:(j + 1) * P * CH, :]])

            def transpose_kc(src_ap, dst_tile):
                """[128, F] node-major -> [128, KC, 128] feature-major."""
                tps = ppool.tile([128, KC, 128], BF16D, tag="tps")
                for k in range(KC):
                    nc.tensor.transpose(
                        tps[:, k, :], src_ap[:, k * 128:(k + 1) * 128], id_sb[:])
                nc.vector.tensor_copy(dst_tile, tps[:])

            import os
            PH = int(os.environ.get("GNN_PHASES", "9"))

            # ---- layer 1: h1 = celu((Ag @ x) @ W1 + b1); x replicated so
            # both source halves are available immediately.
            for t in range(NDT):
                ps = ppool3.tile([128, F], FP32, tag="ps")
                scatter_into(ps, x_bf[0:HALF, :], "ga", t, F, True, False, "ms")
                scatter_into(ps, x_bf[HALF:NTOT, :], "gb", t, F, False, True, "ms")
                agg = wpool.tile([128, F], BF16D, tag="agg")
                nc.vector.tensor_copy(agg[:], ps[:])
                aggT = wpool.tile([128, KC, 128], BF16D, tag="aggT")
                transpose_kc(agg, aggT[:])
                z = ppoolz.tile([128, F], FP32, tag="z")
                for k in range(KC):
                    nc.tensor.matmul(
                        z[:], aggT[:, k, :], w1_sb[:, k * F:(k + 1) * F],
                        start=(k == 0), stop=(k == KC - 1))
                gemm_bias(z[:], F, 0)
                h = wpool.tile([128, F], BF16D, tag="h")
                celu(z[:], F, h[:])
                nc.sync.dma_start(h1c[t * 128:(t + 1) * 128, :], h[:])
                if PH >= 2 and (t + 1) % (NDT // NCH) == 0:
                    allgather_chunk(h1c, h1f, (t + 1) // (NDT // NCH) - 1)

            # ---- layer 2 step A: aggregate half-A sources of h1
            if PH >= 3:
                for t in range(NDT):
                    ps = ppool3.tile([128, F], FP32, tag="ps")
                    scatter_into(ps, h1f[0:HALF, :], "ga", t, F, True, True, "ms")
                    nc.vector.tensor_copy(aggA[:, t, :], ps[:])

            # ---- layer 2 step B + cheb head:
            # h2 = celu((aggA+aggB) @ W2 + b2)
            # y12 = h2 @ [W1c|W2c] -> DRAM (+AG);  acc0 = h2 @ (W0c-W2c) + bc
            if PH >= 4:
                for t in range(NDT):
                    ps = ppool3.tile([128, F], FP32, tag="ps")
                    scatter_into(ps, h1f[HALF:NTOT, :], "gb", t, F, True, True, "ms")
                    agg = wpool.tile([128, F], BF16D, tag="agg")
                    nc.vector.tensor_tensor(agg[:], ps[:], aggA[:, t, :], Alu.add)
                    aggT = wpool.tile([128, KC, 128], BF16D, tag="aggT")
                    transpose_kc(agg, aggT[:])
                    z = ppoolz.tile([128, F], FP32, tag="z")
                    for k in range(KC):
                        nc.tensor.matmul(
                            z[:], aggT[:, k, :], w2_sb[:, k * F:(k + 1) * F],
                            start=(k == 0), stop=(k == KC - 1))
                    gemm_bias(z[:], F, F)
                    h2 = wpool.tile([128, F], BF16D, tag="h")
                    celu(z[:], F, h2[:])
                    h2T = wpool.tile([128, KC, 128], BF16D, tag="h2T")
                    transpose_kc(h2, h2T[:])
                    y12 = ppoolz.tile([128, F], FP32, tag="z")
                    for k in range(KC):
                        nc.tensor.matmul(
                            y12[:], h2T[:, k, :], wcat_sb[:, k * F:(k + 1) * F],
                            start=(k == 0), stop=(k == KC - 1))
                    y12s = wpool.tile([128, F], BF16D, tag="y12s")
                    nc.vector.tensor_copy(y12s[:], y12[:])
                    nc.sync.dma_start(y12c[t * 128:(t + 1) * 128, :], y12s[:])
                    acc = ppoolz.tile([128, F], FP32, tag="z")
                    for k in range(KC):
                        nc.tensor.matmul(
                            acc[:, :DOUT], h2T[:, k, :],
                            wa_sb[:, k * DOUT:(k + 1) * DOUT],
                            start=(k == 0), stop=(k == KC - 1))
                    gemm_bias(acc[:, :DOUT], DOUT, 2 * F)
                    nc.vector.tensor_copy(acc0k[:, t, :], acc[:, :DOUT])
                    if PH >= 5 and (t + 1) % (NDT // NCH) == 0:
                        allgather_chunk(y12c, y12f, (t + 1) // (NDT // NCH) - 1)

            # ---- cheb hop 1 step A on y12
            if PH >= 6:
                for t in range(NDT):
                    ps = ppool3.tile([128, F], FP32, tag="ps")
                    scatter_into(ps, y12f[0:HALF, :], "ca", t, F, True, True, "ms")
                    nc.vector.tensor_copy(aggA1[:, t, :], ps[:])

            # ---- cheb hop 1 step B: [g1 | z2] = lhat([y1 | y2]); z2 -> AG
            if PH >= 7:
                for t in range(NDT):
                    ps = ppool3.tile([128, F], FP32, tag="ps")
                    scatter_into(ps, y12f[HALF:NTOT, :], "cb", t, F, True, True, "ms")
                    nc.vector.tensor_tensor(
                        g1z2k[:, t, :], ps[:], aggA1[:, t, :], Alu.add)
                    nc.sync.dma_start(z2c[t * 128:(t + 1) * 128, :],
                                      g1z2k[:, t, DOUT:F])
                    if PH >= 8 and (t + 1) % (NDT // NCH) == 0:
                        allgather_chunk(z2c, z2f, (t + 1) // (NDT // NCH) - 1)

            # ---- cheb hop 2 step A on z2
            if PH >= 9:
                for t in range(NDT):
                    ps = ppool3.tile([128, F], FP32, tag="ps")
                    scatter_into(ps, z2f[0:HALF, :], "ca", t, DOUT, True, True, "ms2")
                    nc.vector.tensor_copy(aggA2[:, t, :], ps[:, :DOUT])

                # ---- cheb hop 2 step B + output:
                # out = celu(acc0 + g1 + 2*(aggA2+aggB2))
                for t in range(NDT):
                    ps = ppool3.tile([128, F], FP32, tag="ps")
                    scatter_into(ps, z2f[HALF:NTOT, :], "cb", t, DOUT, True, True,
                                 "ms2")
                    g2 = wpool.tile([128, DOUT], FP32, tag="g2")
                    nc.vector.tensor_tensor(
                        g2[:], ps[:, :DOUT], aggA2[:, t, :], Alu.add)
                    s2 = wpool.tile([128, DOUT], FP32, tag="s2")
                    nc.vector.tensor_tensor(
                        s2[:], acc0k[:, t, :], g1z2k[:, t, 0:DOUT], Alu.add)
                    zf = wpool.tile([128, DOUT], FP32, tag="zf")
                    nc.vector.scalar_tensor_tensor(
                        zf[:], g2[:], 2.0, s2[:], Alu.mult, Alu.add)
                    of = wpool.tile([128, DOUT], FP32, tag="of")
                    celu(zf[:], DOUT, of[:])
                    nc.sync.dma_start(outp[t * 128:(t + 1) * 128, :], of[:])

    nc.compile()
    _CACHE[key] = nc
    return nc


# ------------------------------------------------------------------- driver

def _run(inputs, trace=False, tmpdir=None):
    ETs, biases, in_maps = _prep(**inputs)
    has_bias = any(np.any(b != 0) for b in biases)
    if has_bias:
        brow = np.concatenate(biases).astype(np.float32)[None, :]
        for m in in_maps:
            m["brows"] = brow
    nc = _build_program(ETs, has_bias)
    res = run_bass_kernel_spmd(nc, in_maps, core_ids=list(range(P)),
                               trace=trace, tmpdir=tmpdir)
    out = np.concatenate(
        [res.results[c]["out"][:NPC] for c in range(P)], axis=0)
    return out.astype(np.float32), res


def kernel(**inputs) -> np.ndarray:
    out, _ = _run(inputs)
    return out


# revision 18
# speedup vs baseline: 1.2136x; 1.0554x over previous
"""Trainium2 Bass kernel for the ChebConv GNN problem
(nn_ChebConvConvolutional): 2x GCNConv + 1x ChebConv(K=3), N=10000 nodes,
E=160000 edges, F=512, celu activations.

Strategy (8 NeuronCores, SPMD):
  * Nodes are sharded 1250/core (padded to 1280). Edges are sharded by
    destination core and grouped into 128-dest tiles; per dest-tile the
    source nodes are deduplicated and the edge weights are baked into dense
    [128 src x 128 dst] one-hot "S" matrices (GCN self-loops folded in as
    edges with value dinv^2, Cheb normalization negated so the scatter
    directly produces lhat).
  * Pipelined AllGather: every aggregation's sources are split into two
    halves (local rows 0-639 / 640-1279 of each rank). The producing layer
    issues the AG of chunk 0 after its 5th dest tile, chunk 1 at the end;
    the consuming layer first processes all half-A source blocks (needs
    only chunk 0), then half-B, so collectives overlap compute instead of
    stalling the PE (which also avoids HAM cold-clock restarts).
  * ChebConv is computed transform-first:
        y1 = h2@W1, y2 = h2@W2, acc0 = h2@(W0-W2) + bc
        out = celu(acc0 + lhat(y1) + 2*lhat(lhat(y2)))
    so the second-hop aggregate and its AllGather are only 256 wide, and
    no node-major T_k tensors are materialized or transposed.
"""
import numpy as np
import ml_dtypes

import concourse.bacc as bacc
import concourse.mybir as mybir
import concourse.tile as tile
from concourse import library_config
from concourse.bass_utils import run_bass_kernel_spmd
from concourse.tile import add_dep_helper

BF16 = ml_dtypes.bfloat16
FP32 = mybir.dt.float32
BF16D = mybir.dt.bfloat16
I16 = mybir.dt.int16

P = 8            # cores
N = 10000        # nodes
NPC = N // P     # nodes per core
NPAD = 1280      # padded nodes per core
NTOT = NPAD * P
F = 512          # feature width of x / h1 / h2 / [y1|y2]
DOUT = 256
DT = 128         # dests per dest tile
NDT = NPAD // DT # dest tiles per core
KC = F // 128    # contraction chunks (4)
NCH = 2          # AllGather chunks per layer (source-split pipelining)
CH = NPAD // NCH # local rows per AG chunk (640)
HALF = P * CH    # global padded rows per source half (5120)


# ----------------------------------------------------------------- host prep

def _to_padded_id(n):
    """Global node id -> row in the chunked-AllGather global layout:
    [NCH chunks][P ranks][CH rows]."""
    r = n // NPC
    l = n % NPC
    j = l // CH
    return j * (P * CH) + r * CH + (l % CH)


def _build_edge_tiles_split(src, dst, val, three_way=False):
    """Shard by dest core, tile by 128 dests, dedup sources per tile, then
    split each tile's sources by AG half (padded id </>= HALF). With
    three_way, sources owned by the dest core are pulled out first (group L,
    with both local-row and global-padded ids) — they can be aggregated from
    the core's own node-major buffer before any AllGather lands.
    Returns a list of groups, each (ET [NDT], idx(s), S [P, T, 128, DT]):
      [(ETL, idxL_local, idxL_global, SL)] if three_way, then
      (ETA, idxA, SA), (ETB, idxB, SB)."""
    order = np.argsort(dst, kind="stable")
    src, dst, val = src[order], dst[order], val[order]
    core_of = dst // NPC
    core_starts = np.searchsorted(core_of, np.arange(P + 1))
    per_core = []
    ngr = 3 if three_way else 2
    for c in range(P):
        lo, hi = core_starts[c], core_starts[c + 1]
        s, d, v = src[lo:hi], dst[lo:hi] - c * NPC, val[lo:hi]
        tile_of = d // DT
        tile_starts = np.searchsorted(tile_of, np.arange(NDT + 1))
        groups = []
        for t in range(NDT):
            a, b = tile_starts[t], tile_starts[t + 1]
            st, dl, vt = s[a:b], d[a:b] - t * DT, v[a:b]
            uniq, inv = np.unique(st, return_inverse=True)
            zero = (np.zeros(1, np.int64), np.zeros((1, DT), np.float32))
            gs = []
            if len(uniq) == 0:
                gs = [zero] * ngr
            else:
                S = np.zeros((len(uniq), DT), np.float32)
                np.add.at(S, (inv, dl), vt)
                pid = _to_padded_id(uniq)
                loc = (uniq // NPC) == c
                if three_way:
                    if np.any(loc):
                        # local-row id in the core's node-major buffer; the
                        # global padded id rides along for layer-1 (x gather)
                        l = uniq[loc] % NPC
                        gs.append((l, pid[loc], S[loc]))
                    else:
                        gs.append((zero[0], zero[0], zero[1]))
                rem = ~loc if three_way else np.ones(len(uniq), bool)
                for h in range(2):
                    m = rem & ((pid < HALF) if h == 0 else (pid >= HALF))
                    if not np.any(m):
                        gs.append(zero)
                    else:
                        gs.append((pid[m] - h * HALF, S[m]))
            groups.append(gs)
        per_core.append(groups)

    out = []
    for g in range(ngr):
        ET = [max(max((len(per_core[c][t][g][0]) + 127) // 128, 1)
                  for c in range(P)) for t in range(NDT)]
        T = sum(ET)
        off = np.cumsum([0] + ET[:-1])
        nidx = 2 if (three_way and g == 0) else 1
        idxs = [np.zeros((P, T, 128), np.int32) for _ in range(nidx)]
        S_all = np.zeros((P, T, 128, DT), np.float32)
        for c in range(P):
            for t in range(NDT):
                entry = per_core[c][t][g]
                ids_list, S = entry[:-1], entry[-1]
                n = len(ids_list[0])
                o = off[t]
                for i, ids in enumerate(ids_list):
                    idxs[i][c, o:o + (n + 127) // 128].reshape(-1)[:n] = ids
                S_all[c, o:o + (n + 127) // 128].reshape(-1, DT)[:n] = S
        out.append((tuple(ET), *idxs, S_all))
    return out


def _idx_dev(idx_core):
    """[T, 128] int32 -> [128, T*8] int16 (wrap 16 partitions, replicate x8)."""
    flat = idx_core.reshape(-1)
    n = len(flat)
    a = np.zeros((16, n // 16), np.int16)
    a[np.arange(n) % 16, np.arange(n) // 16] = flat.astype(np.int16)
    return np.tile(a, (8, 1))


def _s_dev(S_core):
    """[T, 128, DT] -> [128, T*DT] bf16."""
    T = S_core.shape[0]
    return np.ascontiguousarray(
        S_core.transpose(1, 0, 2).reshape(128, T * DT)).astype(BF16)


def _w_dev(W):
    """[F, fo] -> [128, KC*fo] bf16 (chunk k at cols [k*fo, (k+1)*fo))."""
    fi, fo = W.shape
    k = fi // 128
    return np.ascontiguousarray(
        W.reshape(k, 128, fo).transpose(1, 0, 2).reshape(128, k * fo)).astype(BF16)


def _prep(x, edge_index, edge_weight, W1, b1, W2, b2, Wc, bc):
    row = np.asarray(edge_index[0], np.int64)
    col = np.asarray(edge_index[1], np.int64)
    w = np.asarray(edge_weight, np.float32)

    # GCN norm (layers 1 & 2): deg over dest (col) + 1 self loop.
    deg = np.zeros(N, np.float32)
    np.add.at(deg, col, w)
    deg += 1.0
    dinv = (1.0 / np.sqrt(deg)).astype(np.float32)
    g_src = np.concatenate([row, np.arange(N)])
    g_dst = np.concatenate([col, np.arange(N)])
    g_val = np.concatenate([dinv[row] * w * dinv[col], dinv * dinv]).astype(np.float32)

    # Cheb: drop self loops, deg over src (row), negate (lhat = -A_norm).
    keep = row != col
    r0, c0, w0 = row[keep], col[keep], w[keep]
    deg2 = np.zeros(N, np.float32)
    np.add.at(deg2, r0, w0)
    dinv2 = np.where(deg2 > 0, 1.0 / np.sqrt(deg2), 0.0).astype(np.float32)
    c_val = -(dinv2[r0] * w0 * dinv2[c0]).astype(np.float32)

    ((ETGL, idxgl, idxglx, Sgl), (ETGA, idxga, Sga),
     (ETGB, idxgb, Sgb)) = _build_edge_tiles_split(
        g_src, g_dst, g_val, three_way=True)
    (ETCA, idxca, Sca), (ETCB, idxcb, Scb) = _build_edge_tiles_split(
        r0, c0, c_val)

    x = np.asarray(x, np.float32)
    x_pad = np.zeros((NTOT, F), BF16)
    x_pad[_to_padded_id(np.arange(N))] = x.astype(BF16)

    Wc = np.asarray(Wc, np.float32)
    com = dict(
        x_bf=x_pad,
        w1=_w_dev(np.asarray(W1, np.float32)),
        w2=_w_dev(np.asarray(W2, np.float32)),
        wcat=_w_dev(np.concatenate([Wc[1], Wc[2]], axis=1)),
        wa=_w_dev(Wc[0] - Wc[2]),
        ident=np.eye(128, dtype=BF16),
    )
    biases = (np.asarray(b1, np.float32), np.asarray(b2, np.float32),
              np.asarray(bc, np.float32))
    in_maps = []
    for c in range(P):
        m = dict(com)
        m["idxgl"] = _idx_dev(idxgl[c])
        m["idxglx"] = _idx_dev(idxglx[c])
        m["sgl"] = _s_dev(Sgl[c])
        m["idxga"] = _idx_dev(idxga[c])
        m["sga"] = _s_dev(Sga[c])
        m["idxgb"] = _idx_dev(idxgb[c])
        m["sgb"] = _s_dev(Sgb[c])
        m["idxca"] = _idx_dev(idxca[c])
        m["sca"] = _s_dev(Sca[c])
        m["idxcb"] = _idx_dev(idxcb[c])
        m["scb"] = _s_dev(Scb[c])
        in_maps.append(m)
    ETs = (ETGL, ETGA, ETGB, ETCA, ETCB)
    return ETs, biases, in_maps


# ------------------------------------------------------------- bass program

_CACHE = {}


def _build_program(ETs, has_bias):
    import os
    key = (ETs, has_bias, os.environ.get("GNN_PHASES", "9"))
    if key in _CACHE:
        return _CACHE[key]
    ETGL, ETGA, ETGB, ETCA, ETCB = ETs
    ETMAX = max(max(e) for e in ETs)

    nc = bacc.Bacc("TRN2", target_bir_lowering=False, num_devices=P,
                   num_swdge_queues=4)
    x_bf = nc.dram_tensor("x_bf", [NTOT, F], BF16D, kind="ExternalInput")
    srcs = {}
    for nm, ET in (("ga", ETGA), ("gb", ETGB), ("ca", ETCA), ("cb", ETCB),
                   ("gl", ETGL)):
        T = sum(ET)
        srcs["idx" + nm] = nc.dram_tensor(
            "idx" + nm, [128, T * 8], I16, kind="ExternalInput")
        srcs["s" + nm] = nc.dram_tensor(
            "s" + nm, [128, T * DT], BF16D, kind="ExternalInput")
    srcs["idxglx"] = nc.dram_tensor(
        "idxglx", [128, sum(ETGL) * 8], I16, kind="ExternalInput")
    w1 = nc.dram_tensor("w1", [128, KC * F], BF16D, kind="ExternalInput")
    w2 = nc.dram_tensor("w2", [128, KC * F], BF16D, kind="ExternalInput")
    wcat = nc.dram_tensor("wcat", [128, KC * F], BF16D, kind="ExternalInput")
    wa = nc.dram_tensor("wa", [128, KC * DOUT], BF16D, kind="ExternalInput")
    ident = nc.dram_tensor("ident", [128, 128], BF16D, kind="ExternalInput")
    if has_bias:
        brows = nc.dram_tensor("brows", [1, 2 * F + DOUT], FP32, kind="ExternalInput")
    outp = nc.dram_tensor("out", [NPAD, DOUT], FP32, kind="ExternalOutput")

    warm_i = nc.dram_tensor("warm_i", [1, 128], BF16D, kind="Internal")
    warm_o = nc.dram_tensor("warm_o", [P, 128], BF16D, kind="Internal",
                            addr_space="Shared")
    h1c = nc.dram_tensor("h1c", [NPAD, F], BF16D, kind="Internal")
    h1f = nc.dram_tensor("h1f", [NTOT, F], BF16D, kind="Internal", addr_space="Shared")
    y12c = nc.dram_tensor("y12c", [NPAD, F], BF16D, kind="Internal")
    y12f = nc.dram_tensor("y12f", [NTOT, F], BF16D, kind="Internal",
                          addr_space="Shared")
    z2c = nc.dram_tensor("z2c", [NPAD, DOUT], BF16D, kind="Internal")
    z2f = nc.dram_tensor("z2f", [NTOT, DOUT], BF16D, kind="Internal",
                         addr_space="Shared")

    Exp = mybir.ActivationFunctionType.Exp
    Alu = mybir.AluOpType

    offs = {nm: np.cumsum([0] + list(ET[:-1]))
            for nm, ET in (("ga", ETGA), ("gb", ETGB), ("ca", ETCA),
                           ("cb", ETCB), ("gl", ETGL))}
    offs["glx"] = offs["gl"]
    ETd = {"ga": ETGA, "gb": ETGB, "ca": ETCA, "cb": ETCB,
           "gl": ETGL, "glx": ETGL}

    with tile.TileContext(nc) as tc:
        with (
            tc.tile_pool(name="const", bufs=1) as cpool,
            tc.tile_pool(name="keep", bufs=1) as kpool,
            tc.tile_pool(name="msgs", bufs=3) as mpool,
            tc.tile_pool(name="work", bufs=3) as wpool,
            tc.tile_pool(name="psum", bufs=2, space="PSUM") as ppool,
            tc.tile_pool(name="psumz", bufs=3, space="PSUM") as ppoolz,
            tc.tile_pool(name="psum3", bufs=3, space="PSUM") as ppool3,
        ):
            lib = nc.gpsimd.load_library(library_config.mlp)

            # Tiny warm-up collective issued first: absorbs the one-time CC
            # init barrier into the load window so the first real AllGather
            # starts at its issue point instead of queueing behind it.
            nc.gpsimd.collective_compute(
                "AllGather", mybir.AluOpType.bypass,
                replica_groups=[list(range(P))],
                ins=[warm_i[0:1, :]], outs=[warm_o[0:P, :]])

            id_sb = cpool.tile([128, 128], BF16D, tag="id")
            nc.sync.dma_start(id_sb[:], ident[:])

            # Per-tile chunked loads of idx + S so tile-0 work starts early.
            idx_sb = {}
            s_sb = {}
            for nm in ("ga", "gb", "ca", "cb", "gl", "glx"):
                T = sum(ETd[nm])
                idx_sb[nm] = cpool.tile([128, T * 8], I16, tag="i" + nm,
                                        name="idx_" + nm)
                if nm != "glx":
                    s_sb[nm] = cpool.tile([128, T * DT], BF16D, tag="s" + nm,
                                          name="s_" + nm)
            s_sb["glx"] = s_sb["gl"]
            for t in range(NDT):
                for nm in ("gl", "glx", "ga", "gb", "ca", "cb"):
                    o, e = offs[nm][t], ETd[nm][t]
                    a, b = o * 8, (o + e) * 8
                    nc.sync.dma_start(idx_sb[nm][:, a:b], srcs["idx" + nm][:, a:b])
                    if nm == "glx":
                        continue
                    a, b = o * DT, (o + e) * DT
                    nc.sync.dma_start(s_sb[nm][:, a:b], srcs["s" + nm][:, a:b])

            w1_sb = cpool.tile([128, KC * F], BF16D, tag="w1")
            nc.sync.dma_start(w1_sb[:], w1[:])
            w2_sb = cpool.tile([128, KC * F], BF16D, tag="w2")
            nc.sync.dma_start(w2_sb[:], w2[:])
            wcat_sb = cpool.tile([128, KC * F], BF16D, tag="wcat")
            nc.sync.dma_start(wcat_sb[:], wcat[:])
            wa_sb = cpool.tile([128, KC * DOUT], BF16D, tag="wa")
            nc.sync.dma_start(wa_sb[:], wa[:])
            if has_bias:
                br_sb = cpool.tile([1, 2 * F + DOUT], FP32, tag="br")
                nc.sync.dma_start(br_sb[:], brows[:])
                ones_sb = cpool.tile([1, 128], FP32, tag="ones")
                nc.vector.memset(ones_sb[:], 1.0)

            # step-A aggregates kept across the AG boundary. One buffer,
            # reused by L2 / cheb hop 1 / cheb hop 2 (per-tile WAR deps
            # follow the pipeline order, so reuse costs no stalls).
            aggA = kpool.tile([128, NDT, F], BF16D, tag="aggA")
            aggA1 = aggA
            aggA2 = aggA[:, :, :DOUT]
            acc0k = kpool.tile([128, NDT, DOUT], BF16D, tag="acc0k")
            g1z2k = kpool.tile([128, NDT, F], BF16D, tag="g1z2k")

            first_gather = [0]
            qctr = [0]

            def scatter_into(ps, src_ap, nm, t, width, start, stop, mtag):
                """Gather half-`nm` sources of dest-tile t and accumulate
                their one-hot matmuls into psum `ps` ([128, width])."""
                o = offs[nm][t]
                et = ETd[nm][t]
                msgs = mpool.tile([128, ETMAX, width], BF16D, tag=mtag)
                isb = idx_sb[nm]
                ssb = s_sb[nm]
                nq = min(4, et)
                bounds = [et * i // nq for i in range(nq + 1)]
                for a, b in zip(bounds[:-1], bounds[1:]):
                    if b <= a:
                        continue
                    q = qctr[0] % 4
                    qctr[0] += 1
                    gi = nc.gpsimd.dma_gather(
                        msgs[:, a:b, :], src_ap,
                        isb[:, (o + a) * 8:(o + b) * 8],
                        (b - a) * 128, (b - a) * 128, width,
                        single_packet=False, queue_num=q)
                    if first_gather[0] < 4:
                        add_dep_helper(gi.ins, lib.ins,
                                       reason="mlp lib before gather")
                        first_gather[0] += 1
                for g in range(et):
                    nc.tensor.matmul(
                        ps[:, :width],
                        ssb[:, (o + g) * DT:(o + g + 1) * DT],
                        msgs[:, g, :],
                        start=(start and g == 0), stop=(stop and g == et - 1))

            def celu(z_ap, width, out_ap):
                """out = max(z,0) + min(exp(z)-1, 0)."""
                e = wpool.tile([128, F], FP32, tag="e")
                nc.scalar.activation(e[:, :width], z_ap, Exp)
                nc.vector.tensor_scalar(
                    e[:, :width], e[:, :width], 1.0, 0.0,
                    Alu.subtract, Alu.min)
                nc.vector.scalar_tensor_tensor(
                    out_ap, z_ap, 0.0, e[:, :width], Alu.max, Alu.add)

            def gemm_bias(z_ps, width, b_off):
                if has_bias:
                    nc.tensor.matmul(
                        z_ps, ones_sb[:],
                        br_sb[:, b_off:b_off + width],
                        start=False, stop=False)

            def allgather_chunk(cin, cout, j):
                nc.gpsimd.collective_compute(
                    "AllGather", Alu.bypass,
                    replica_groups=[list(range(P))],
                    ins=[cin[j * CH:(j + 1) * CH, :]],
                    outs=[cout[j * P * CH:(j + 1) * P * CH, :]])

            def transpose_kc(src_ap, dst_tile):
                """[128, F] node-major -> [128, KC, 128] feature-major."""
                tps = ppool.tile([128, KC, 128], BF16D, tag="tps")
                for k in range(KC):
                    nc.tensor.transpose(
                        tps[:, k, :], src_ap[:, k * 128:(k + 1) * 128], id_sb[:])
                nc.vector.tensor_copy(dst_tile, tps[:])

            import os
            PH = int(os.environ.get("GNN_PHASES", "9"))

            # ---- layer 1: h1 = celu((Ag @ x) @ W1 + b1); x replicated so
            # both source halves are available immediately.
            for t in range(NDT):
                ps = ppool3.tile([128, F], FP32, tag="ps")
                scatter_into(ps, x_bf[:, :], "glx", t, F, True, False, "ms")
                scatter_into(ps, x_bf[0:HALF, :], "ga", t, F, False, False, "ms")
                scatter_into(ps, x_bf[HALF:NTOT, :], "gb", t, F, False, True, "ms")
                agg = wpool.tile([128, F], BF16D, tag="agg")
                nc.vector.tensor_copy(agg[:], ps[:])
                aggT = wpool.tile([128, KC, 128], BF16D, tag="aggT")
                transpose_kc(agg, aggT[:])
                z = ppoolz.tile([128, F], FP32, tag="z")
                for k in range(KC):
                    nc.tensor.matmul(
                        z[:], aggT[:, k, :], w1_sb[:, k * F:(k + 1) * F],
                        start=(k == 0), stop=(k == KC - 1))
                gemm_bias(z[:], F, 0)
                h = wpool.tile([128, F], BF16D, tag="h")
                celu(z[:], F, h[:])
                nc.sync.dma_start(h1c[t * 128:(t + 1) * 128, :], h[:])
                if PH >= 2 and (t + 1) % (NDT // NCH) == 0:
                    allgather_chunk(h1c, h1f, (t + 1) // (NDT // NCH) - 1)

            # ---- layer 2 step L: aggregate this core's own h1 rows (no AG
            # dependency — fills the collective-init window after layer 1)
            if PH >= 3:
                for t in range(NDT):
                    ps = ppool3.tile([128, F], FP32, tag="ps")
                    scatter_into(ps, h1c[:, :], "gl", t, F, True, True, "ms")
                    nc.vector.tensor_copy(aggA[:, t, :], ps[:])

                # ---- layer 2 step A: half-A remote sources (needs AG chunk 0)
                for t in range(NDT):
                    ps = ppool3.tile([128, F], FP32, tag="ps")
                    scatter_into(ps, h1f[0:HALF, :], "ga", t, F, True, True, "ms")
                    nc.vector.tensor_tensor(
                        aggA[:, t, :], ps[:], aggA[:, t, :], Alu.add)

            # ---- layer 2 step B + cheb head:
            # h2 = celu((aggA+aggB) @ W2 + b2)
            # y12 = h2 @ [W1c|W2c] -> DRAM (+AG);  acc0 = h2 @ (W0c-W2c) + bc
            if PH >= 4:
                for t in range(NDT):
                    ps = ppool3.tile([128, F], FP32, tag="ps")
                    scatter_into(ps, h1f[HALF:NTOT, :], "gb", t, F, True, True, "ms")
                    agg = wpool.tile([128, F], BF16D, tag="agg")
                    nc.vector.tensor_tensor(agg[:], ps[:], aggA[:, t, :], Alu.add)
                    aggT = wpool.tile([128, KC, 128], BF16D, tag="aggT")
                    transpose_kc(agg, aggT[:])
                    z = ppoolz.tile([128, F], FP32, tag="z")
                    for k in range(KC):
                        nc.tensor.matmul(
                            z[:], aggT[:, k, :], w2_sb[:, k * F:(k + 1) * F],
                            start=(k == 0), stop=(k == KC - 1))
                    gemm_bias(z[:], F, F)
                    h2 = wpool.tile([128, F], BF16D, tag="h")
                    celu(z[:], F, h2[:])
                    h2T = wpool.tile([128, KC, 128], BF16D, tag="h2T")
                    transpose_kc(h2, h2T[:])
                    y12 = ppoolz.tile([128, F], FP32, tag="z")
                    for k in range(KC):
                        nc.tensor.matmul(
                            y12[:], h2T[:, k, :], wcat_sb[:, k * F:(k + 1) * F],
                            start=(k == 0), stop=(k == KC - 1))
                    y12s = wpool.tile([128, F], BF16D, tag="y12s")
                    nc.vector.tensor_copy(y12s[:], y12[:])
                    nc.sync.dma_start(y12c[t * 128:(t + 1) * 128, :], y12s[:])
                    acc = ppoolz.tile([128, F], FP32, tag="z")
                    for k in range(KC):
                        nc.tensor.matmul(
                            acc[:, :DOUT], h2T[:, k, :],
                            wa_sb[:, k * DOUT:(k + 1) * DOUT],
                            start=(k == 0), stop=(k == KC - 1))
                    gemm_bias(acc[:, :DOUT], DOUT, 2 * F)
                    nc.vector.tensor_copy(acc0k[:, t, :], acc[:, :DOUT])
                    if PH >= 5 and (t + 1) % (NDT // NCH) == 0:
                        allgather_chunk(y12c, y12f, (t + 1) // (NDT // NCH) - 1)

            # ---- cheb hop 1 step A on y12
            if PH >= 6:
                for t in range(NDT):
                    ps = ppool3.tile([128, F], FP32, tag="ps")
                    scatter_into(ps, y12f[0:HALF, :], "ca", t, F, True, True, "ms")
                    nc.vector.tensor_copy(aggA1[:, t, :], ps[:])

            # ---- cheb hop 1 step B: [g1 | z2] = lhat([y1 | y2]); z2 -> AG
            if PH >= 7:
                for t in range(NDT):
                    ps = ppool3.tile([128, F], FP32, tag="ps")
                    scatter_into(ps, y12f[HALF:NTOT, :], "cb", t, F, True, True, "ms")
                    nc.vector.tensor_tensor(
                        g1z2k[:, t, :], ps[:], aggA1[:, t, :], Alu.add)
                    nc.sync.dma_start(z2c[t * 128:(t + 1) * 128, :],
                                      g1z2k[:, t, DOUT:F])
                    if PH >= 8 and (t + 1) % (NDT // NCH) == 0:
                        allgather_chunk(z2c, z2f, (t + 1) // (NDT // NCH) - 1)

            # ---- cheb hop 2 step A on z2
            if PH >= 9:
                for t in range(NDT):
                    ps = ppool3.tile([128, F], FP32, tag="ps")
                    scatter_into(ps, z2f[0:HALF, :], "ca", t, DOUT, True, True, "ms2")
                    nc.vector.tensor_copy(aggA2[:, t, :], ps[:, :DOUT])

                # ---- cheb hop 2 step B + output:
                # out = celu(acc0 + g1 + 2*(aggA2+aggB2))
                for t in range(NDT):
                    ps = ppool3.tile([128, F], FP32, tag="ps")
                    scatter_into(ps, z2f[HALF:NTOT, :], "cb", t, DOUT, True, True,
                                 "ms2")
                    g2 = wpool.tile([128, DOUT], FP32, tag="g2")
                    nc.vector.tensor_tensor(
                        g2[:], ps[:, :DOUT], aggA2[:, t, :], Alu.add)
                    s2 = wpool.tile([128, DOUT], FP32, tag="s2")
                    nc.vector.tensor_tensor(
                        s2[:], acc0k[:, t, :], g1z2k[:, t, 0:DOUT], Alu.add)
                    zf = wpool.tile([128, DOUT], FP32, tag="zf")
                    nc.vector.scalar_tensor_tensor(
                        zf[:], g2[:], 2.0, s2[:], Alu.mult, Alu.add)
                    of = wpool.tile([128, DOUT], FP32, tag="of")
                    celu(zf[:], DOUT, of[:])
                    nc.sync.dma_start(outp[t * 128:(t + 1) * 128, :], of[:])

    nc.compile()
    _CACHE[key] = nc
    return nc


# ------------------------------------------------------------------- driver

def _run(inputs, trace=False, tmpdir=None):
    ETs, biases, in_maps = _prep(**inputs)
    has_bias = any(np.any(b != 0) for b in biases)
    if has_bias:
        brow = np.concatenate(biases).astype(np.float32)[None, :]
        for m in in_maps:
            m["brows"] = brow
    nc = _build_program(ETs, has_bias)
    res = run_bass_kernel_spmd(nc, in_maps, core_ids=list(range(P)),
                               trace=trace, tmpdir=tmpdir)
    out = np.concatenate(
        [res.results[c]["out"][:NPC] for c in range(P)], axis=0)
    return out.astype(np.float32), res


def kernel(**inputs) -> np.ndarray:
    out, _ = _run(inputs)
    return out
